# revision 31
# baseline (speedup 1.0000x reference)
"""GAT (2-layer) on 8 NeuronCores — Bass/Tile kernel.

Strategy (dst-sharded graph parallel):
  - Each core owns 12500 destination nodes, degree-sorted and tiled into
    128-dst tiles on a dense slot grid (slot width = exact per-tile max
    degree over all cores); adjacent similar-degree tiles are merged into
    <=128-column calls (Dmax/Dmin <= 1.25 slack).
  - Host pre-pass is index-only: the slot grid, per-slot source-node ids
    (esrc), pad masks, permutations.
  - Launch A: per-core [Wh1^T; s_i; s_j] = ([W1 | W1@A1]^T x^T + bias) in a
    single fused fp16 matmul per 512-column chunk (f32 accumulate).
  - Between launches the host stages per-core inputs by pure indexing of
    device-computed values: slot-expanded fp16 features whe = Wh16[esrc] and
    pad-masked source scalars sje, so the device streams edge data with
    large sequential DMAs at full HBM bandwidth instead of per-edge gather
    descriptors.
  - Launch B (x2, one per GAT layer), phase 1 computes every attention
    weight up front: e = leaky(s_i + s_j + bA) on DVE+ACT over the whole
    slot grid, ex = exp(e) in fp16 (scores are O(5), no max-subtract
    needed), per-call denominator reduce, reciprocal x zero-degree flags.
    Phase 2 streams calls through a software pipeline: slot-chunk DMA ->
    message multiply split ~50/50 between the gpsimd and vector engines ->
    fp16 pairwise-tree slot reduction (2x DVE mode) -> per-tile scalar-engine
    Prelu fusing the softmax normalization (scale=1/den) with the leaky
    relu -> fp16 PE transpose -> fused epilogue matmul [Wn | Wn@An] giving
    the next layer's [Wh^T; s_i; s_j] (or the final fc output for layer 2).
"""

import dataclasses
import numpy as np

import concourse.bacc as bacc
import concourse.tile as tile
from concourse import bass, mybir, bass_utils
from concourse.masks import make_identity

F32 = mybir.dt.float32
F16 = mybir.dt.float16

N_NODES = 100000
N_CORES = 8
DPC = N_NODES // N_CORES
F = 64
IN_C = 128
NSUB = 1
CALL_W = 128  # max slot-columns per vector-op call
MERGE_SLACK = 1.25  # max Dmax/Dmin when merging tiles into one call
POOL_MULT_SHARE = 0.5  # fraction of message-multiply elems on gpsimd
NEG_BIG = -1.0e30
EPS = 1e-16
ALPHA = 0.2


@dataclasses.dataclass
class Schedule:
    n_tiles: int
    w_total: int
    calls: list  # (sub, t0, ntc, D, col0)
    tilecol: np.ndarray  # int32 [w_total]: tile index of each slot column
    perms: list  # per core: int64 [n_tiles*128], local dst or -1
    esrc: list  # per core: int32 [128, w_total] global source id per slot (0 pad)
    emask: list  # per core: f32 [128, w_total] (0 real / NEG_BIG pad)
    flags: list  # per core: f32 [128, n_tiles]


def build_schedule(edge_index: np.ndarray) -> Schedule:
    src = np.asarray(edge_index[0], dtype=np.int64)
    dst = np.asarray(edge_index[1], dtype=np.int64)
    order = np.argsort(dst, kind="stable")
    src_s = src[order]
    deg_all = np.bincount(dst, minlength=N_NODES)
    starts_all = np.concatenate([[0], np.cumsum(deg_all)])

    # per-core sub-shard dst lists (round-robin over degree-sorted order)
    core_subs = []  # [core][sub] -> local dst ids
    for c in range(N_CORES):
        deg = deg_all[c * DPC : (c + 1) * DPC]
        rank = np.argsort(deg, kind="stable")
        core_subs.append([rank[s::NSUB] for s in range(NSUB)])

    # shared tile plan: per (sub, tile): D = max over cores of tile max-deg
    tiles = []
    for s in range(NSUB):
        nt = -(-max(len(core_subs[c][s]) for c in range(N_CORES)) // 128)
        for t in range(nt):
            mx = 1
            for c in range(N_CORES):
                lst = core_subs[c][s][t * 128 : (t + 1) * 128]
                if len(lst):
                    deg = deg_all[c * DPC + lst]
                    mx = max(mx, int(deg.max()))
            assert mx <= 512, mx  # a tile above CALL_W just becomes its own call
            tiles.append((s, mx))
    n_tiles = len(tiles)

    # call plan: merge consecutive same-sub tiles with similar D
    calls = []
    i = 0
    col = 0
    while i < n_tiles:
        s, D = tiles[i]
        Dmax = Dmin = D
        ntc = 1
        while i + ntc < n_tiles:
            s2, D2 = tiles[i + ntc]
            if s2 != s:
                break
            nD, mD = max(Dmax, D2), min(Dmin, D2)
            if (ntc + 1) * nD > CALL_W or nD > MERGE_SLACK * mD:
                break
            Dmax, Dmin = nD, mD
            ntc += 1
        calls.append((s, i, ntc, Dmax, col))
        col += ntc * Dmax
        i += ntc
    w_total = col
    tilecol = np.zeros(w_total, np.int32)
    for (s_, t0, ntc, D, col0) in calls:
        for tl in range(ntc):
            tilecol[col0 + tl * D : col0 + (tl + 1) * D] = t0 + tl

    perms, esrcs, emasks, flagss = [], [], [], []
    for c in range(N_CORES):
        perm = np.full(n_tiles * 128, -1, dtype=np.int64)
        ti = 0
        for s in range(NSUB):
            nt = sum(1 for (ss, _) in tiles if ss == s)
            lst = core_subs[c][s]
            block = np.full(nt * 128, -1, dtype=np.int64)
            block[: len(lst)] = lst
            perm[ti * 128 : (ti + nt) * 128] = block
            ti += nt

        esrc = np.zeros((128, w_total), np.int32)
        emask = np.full((128, w_total), np.float32(NEG_BIG))
        for (s, t0, ntc, D, col0) in calls:
            for tl in range(ntc):
                tglob = t0 + tl
                dsts = perm[tglob * 128 : (tglob + 1) * 128]
                for p in range(128):
                    d = dsts[p]
                    if d < 0:
                        continue
                    g = c * DPC + d
                    e0, ne = starts_all[g], deg_all[g]
                    c0 = col0 + tl * D
                    esrc[p, c0 : c0 + ne] = src_s[e0 : e0 + ne]
                    emask[p, c0 : c0 + ne] = 0.0
        pflat = perm.copy()
        okdeg = (pflat >= 0) & (deg_all[np.clip(c * DPC + pflat, 0, N_NODES - 1)] > 0)
        flags = np.ascontiguousarray(
            okdeg.reshape(n_tiles, 128).T.astype(np.float32)
        )
        perms.append(perm)
        esrcs.append(esrc)
        emasks.append(emask)
        flagss.append(flags)

    return Schedule(n_tiles, w_total, calls, tilecol, perms, esrcs, emasks, flagss)


# ---------------------------------------------------------------- prog A
def build_progA(n_loc=DPC, in_c=IN_C, f=F):
    nc = bacc.Bacc("TRN2", target_bir_lowering=False, debug=False, num_devices=N_CORES)
    xT = nc.dram_tensor("xT", [in_c, n_loc], F16, kind="ExternalInput").ap()
    Wf = nc.dram_tensor("Wf", [in_c, f + 2], F16, kind="ExternalInput").ap()
    bf = nc.dram_tensor("bf", [f + 2, 1], F32, kind="ExternalInput").ap()
    outT = nc.dram_tensor("outT", [f + 2, n_loc], F16, kind="ExternalOutput").ap()

    AF = mybir.ActivationFunctionType
    OP = mybir.AluOpType

    with tile.TileContext(nc) as tc:
        with tc.tile_pool(name="sb", bufs=1) as pool, tc.tile_pool(
            name="ps", bufs=4, space="PSUM"
        ) as pps, tc.tile_pool(name="sb2", bufs=4) as pool2:
            xT_sb = pool.tile([in_c, n_loc], F16)
            Wf_sb = pool.tile([in_c, f + 2], F16)
            nc.sync.dma_start(out=Wf_sb[:], in_=Wf[:, :])
            bf_sb = pool.tile([f + 2, 1], F32)
            nc.sync.dma_start(out=bf_sb[:], in_=bf[:, :])
            NSPL = 2
            spl = -(-n_loc // NSPL)
            for k in range(NSPL):
                a, b = k * spl, min(n_loc, (k + 1) * spl)
                nc.sync.dma_start(out=xT_sb[:, a:b], in_=xT[:, a:b])

            CH = 512
            GB = 4  # chunks per output DMA
            ob = None
            nch = -(-n_loc // CH)
            for ci, c0 in enumerate(range(0, n_loc, CH)):
                ch = min(CH, n_loc - c0)
                ps_w = pps.tile([f + 2, CH], F32, space="PSUM")
                nc.tensor.matmul(
                    out=ps_w[:, :ch],
                    lhsT=Wf_sb[:],
                    rhs=xT_sb[:, c0 : c0 + ch],
                    start=True,
                    stop=True,
                )
                g = ci % GB
                if g == 0:
                    ob = pool2.tile([f + 2, GB * CH], F16, tag="ob")
                if ci % 2 == 0:
                    nc.scalar.activation(
                        out=ob[:, g * CH : g * CH + ch],
                        in_=ps_w[:, :ch],
                        func=AF.Identity,
                        bias=bf_sb[:],
                    )
                else:
                    nc.vector.tensor_scalar(
                        out=ob[:, g * CH : g * CH + ch],
                        in0=ps_w[:, :ch],
                        scalar1=bf_sb[:],
                        scalar2=None,
                        op0=OP.add,
                    )
                if g == GB - 1 or ci == nch - 1:
                    b0 = (ci - g) * CH
                    nc.sync.dma_start(
                        out=outT[:, b0 : c0 + ch], in_=ob[:, : g * CH + ch]
                    )
    nc.compile()
    return nc


# ---------------------------------------------------------------- prog B
def build_progB(sched: Schedule, f=F):
    NT = sched.n_tiles
    WTOT = sched.w_total
    nc = bacc.Bacc(
        "TRN2",
        target_bir_lowering=False,
        debug=False,
        num_devices=N_CORES,
    )
    whe_d = nc.dram_tensor("whe", [128, WTOT * f], F16, kind="ExternalInput").ap()
    sje_d = nc.dram_tensor("sje", [128, WTOT], F32, kind="ExternalInput").ap()
    si_d = nc.dram_tensor("si", [128, NT], F32, kind="ExternalInput").ap()
    bA_d = nc.dram_tensor("bA", [128, 1], F32, kind="ExternalInput").ap()
    flags_d = nc.dram_tensor("flags", [128, NT], F32, kind="ExternalInput").ap()
    Wf_d = nc.dram_tensor("Wf", [f, f + 2], F16, kind="ExternalInput").ap()
    bf_d = nc.dram_tensor("bf", [f + 2, 1], F32, kind="ExternalInput").ap()
    outT_d = nc.dram_tensor("outT", [f + 2, NT * 128], F16, kind="ExternalOutput").ap()

    X = mybir.AxisListType.X
    AF = mybir.ActivationFunctionType
    OP = mybir.AluOpType

    def v(ap, dims, off=0):
        return dataclasses.replace(
            ap,
            ap=[list(ap.ap[0])] + [list(d) for d in dims],
            offset=ap.offset + off,
        )

    with tile.TileContext(nc) as tc:
        with tc.tile_pool(name="const", bufs=1) as pc, tc.tile_pool(
            name="io", bufs=4
        ) as pio, tc.tile_pool(name="work", bufs=3) as pw, tc.tile_pool(
            name="ps", bufs=2, space="PSUM"
        ) as pps, tc.tile_pool(name="ep", bufs=2) as pep:
            flags_sb = pc.tile([128, NT], F32)
            nc.sync.dma_start(out=flags_sb[:], in_=flags_d[:, :])
            sje_sb = pc.tile([128, WTOT], F32)
            nc.sync.dma_start(out=sje_sb[:], in_=sje_d[:, :])
            si_sb = pc.tile([128, NT], F32)
            nc.sync.dma_start(out=si_sb[:], in_=si_d[:, :])
            bA_sb = pc.tile([128, 1], F32)
            nc.sync.dma_start(out=bA_sb[:], in_=bA_d[:, :])
            Wf_sb = pc.tile([f, f + 2], F16)
            nc.sync.dma_start(out=Wf_sb[:], in_=Wf_d[:, :])
            bf_sb = pc.tile([f + 2, 1], F32)
            nc.sync.dma_start(out=bf_sb[:], in_=bf_d[:, :])
            ident = pc.tile([128, 128], F16)
            make_identity(nc, ident[:])

            # ---- phase 1: attention weights for the whole slot grid ----
            # e = leaky(si + sj + bA) (sj pre-masked to -inf on pad slots);
            # ex = exp(e)  (no max-subtract: scores are O(10) so exp fits
            # fp16/f32 comfortably)
            ep_sb = pc.tile([128, WTOT], F32)
            for (s, t0, ntc, D, col0) in sched.calls:
                nc.vector.tensor_tensor(
                    out=v(ep_sb[:], [(D, ntc), (1, D)], off=col0),
                    in0=v(sje_sb[:], [(D, ntc), (1, D)], off=col0),
                    in1=si_sb[:, t0 : t0 + ntc].to_broadcast([128, ntc, D]),
                    op=OP.add,
                )
            ex16 = pc.tile([128, WTOT], F16)
            nc.scalar.activation(
                out=ep_sb[:], in_=ep_sb[:], func=AF.Prelu, alpha=ALPHA, bias=bA_sb[:]
            )
            nc.scalar.activation(out=ex16[:], in_=ep_sb[:], func=AF.Exp)
            den = pc.tile([128, NT], F32)
            for (s, t0, ntc, D, col0) in sched.calls:
                nc.vector.tensor_reduce(
                    out=den[:, t0 : t0 + ntc],
                    in_=v(ex16[:], [(D, ntc), (1, D)], off=col0),
                    axis=X,
                    op=OP.add,
                )
            rden = pc.tile([128, NT], F32)
            nc.vector.tensor_scalar(
                out=den[:], in0=den[:], scalar1=EPS, scalar2=None, op0=OP.add
            )
            nc.vector.reciprocal(out=rden[:], in_=den[:])
            nc.vector.tensor_tensor(
                out=rden[:], in0=rden[:], in1=flags_sb[:], op=OP.mult
            )

            # ---- phase 2: stream messages, reduce, epilogue ----
            CHT = 4  # tiles per epilogue chunk (512 dsts)

            def flush_chunk(ts, ntl, hTL, ob):
                cols = ntl * 128
                ps_w = pps.tile([f + 2, CHT * 128], F32, tag="psw", space="PSUM")
                nc.tensor.matmul(
                    out=ps_w[:, :cols],
                    lhsT=Wf_sb[:],
                    rhs=hTL[:, :cols],
                    start=True,
                    stop=True,
                )
                nc.scalar.activation(
                    out=ob[:, :cols],
                    in_=ps_w[:, :cols],
                    func=AF.Identity,
                    bias=bf_sb[:],
                )
                nc.sync.dma_start(
                    out=outT_d[:, ts * 128 : ts * 128 + cols],
                    in_=ob[:, :cols],
                )

            def stage1(ci):
                (s, t0, ntc, D, col0) = sched.calls[ci]
                W = ntc * D
                whe = pio.tile([128, W * f], F16, tag="whe")
                cp = max(1, min(W - 1, int(round(POOL_MULT_SHARE * W))))
                nc.sync.dma_start(
                    out=whe[:], in_=whe_d[:, col0 * f : (col0 + W) * f]
                )
                # weighted message (fp16): msg = Wh16 * ex16; each call's
                # multiply is split between gpsimd and DVE at the balance point
                msg16 = pw.tile([128, W * f], F16, tag="msg16")
                nc.gpsimd.tensor_tensor(
                    out=v(msg16[:], [(f, cp), (1, f)]),
                    in0=v(whe[:], [(f, cp), (1, f)]),
                    in1=ex16[:, col0 : col0 + cp].to_broadcast([128, cp, f]),
                    op=OP.mult,
                )
                nc.vector.tensor_tensor(
                    out=v(msg16[:], [(f, W - cp), (1, f)], off=cp * f),
                    in0=v(whe[:], [(f, W - cp), (1, f)], off=cp * f),
                    in1=ex16[:, col0 + cp : col0 + W].to_broadcast(
                        [128, W - cp, f]
                    ),
                    op=OP.mult,
                )
                return msg16

            def stage2(ci, msg16):
                (s, t0, ntc, D, col0) = sched.calls[ci]
                W = ntc * D
                # pairwise-tree reduce over the D slots (fp16 packed -> 2x DVE)
                hraw = pw.tile([128, ntc * f], F32, tag="hraw")
                if D == 1:
                    nc.vector.tensor_copy(
                        out=v(hraw[:], [(f, ntc), (1, f)]),
                        in_=v(msg16[:], [(D * f, ntc), (1, f)]),
                    )
                else:
                    p2 = 1
                    while p2 * 2 <= D:
                        p2 *= 2
                    if D > p2:
                        r = D - p2
                        nc.vector.tensor_tensor(
                            out=v(msg16[:], [(D * f, ntc), (1, r * f)]),
                            in0=v(msg16[:], [(D * f, ntc), (1, r * f)]),
                            in1=v(msg16[:], [(D * f, ntc), (1, r * f)], off=p2 * f),
                            op=OP.add,
                        )
                    while p2 > 2:
                        h = p2 // 2
                        nc.vector.tensor_tensor(
                            out=v(msg16[:], [(D * f, ntc), (1, h * f)]),
                            in0=v(msg16[:], [(D * f, ntc), (1, h * f)]),
                            in1=v(msg16[:], [(D * f, ntc), (1, h * f)], off=h * f),
                            op=OP.add,
                        )
                        p2 = h
                    nc.vector.tensor_tensor(
                        out=v(hraw[:], [(f, ntc), (1, f)]),
                        in0=v(msg16[:], [(D * f, ntc), (1, f)]),
                        in1=v(msg16[:], [(D * f, ntc), (1, f)], off=f),
                        op=OP.add,
                    )
                # h = leaky(hraw * rden): per-tile ACT fuses the softmax
                # normalization (scale) with the leaky relu
                for tl in range(ntc):
                    t = t0 + tl
                    hl = pw.tile([128, f], F16, tag="hl")
                    nc.scalar.activation(
                        out=hl[:],
                        in_=hraw[:, tl * f : (tl + 1) * f],
                        func=AF.Prelu,
                        alpha=ALPHA,
                        scale=rden[:, t : t + 1],
                    )
                    ps_t = pps.tile([f, 128], F16, tag="pst", space="PSUM")
                    nc.tensor.transpose(
                        out=ps_t[:], in_=hl[:], identity=ident[:]
                    )
                    j = tl % CHT
                    if j == 0:
                        hTL = pep.tile([f, CHT * 128], F16, tag="hTL")
                        ob = pep.tile([f + 2, CHT * 128], F16, tag="ob")
                    nc.scalar.activation(
                        out=hTL[:, j * 128 : (j + 1) * 128],
                        in_=ps_t[:],
                        func=AF.Identity,
                    )
                    if j == CHT - 1 or tl == ntc - 1:
                        flush_chunk(t - j, j + 1, hTL, ob)

            # 2-stage software pipeline: issue call ci+1's DMA+multiplies
            # before call ci's reduce/epilogue so the in-order DVE queue
            # never blocks the next multiply behind a Pool-gated reduce
            ncalls = len(sched.calls)
            prev = None
            for ci in range(ncalls):
                cur = stage1(ci)
                if prev is not None:
                    stage2(ci - 1, prev)
                prev = cur
            stage2(ncalls - 1, prev)
    nc.compile()
    return nc


# ---------------------------------------------------------------- driver
_cache = {}
last_results = []  # BassKernelResults per launch (for test.py profiling)


def kernel(x, edge_index, W1, bW1, A1, bA1, W2, bW2, A2, bA2, Wfc, bfc):
    x = np.asarray(x, dtype=np.float32)
    edge_index = np.asarray(edge_index)
    W1 = np.asarray(W1, np.float32)
    bW1 = np.asarray(bW1, np.float32)
    A1 = np.asarray(A1, np.float32)
    bA1 = np.asarray(bA1, np.float32)
    W2 = np.asarray(W2, np.float32)
    bW2 = np.asarray(bW2, np.float32)
    A2 = np.asarray(A2, np.float32)
    bA2 = np.asarray(bA2, np.float32)
    Wfc = np.asarray(Wfc, np.float32)
    bfc = np.asarray(bfc, np.float32)

    sched = build_schedule(edge_index)
    cores = list(range(N_CORES))
    last_results.clear()

    if "A" not in _cache:
        _cache["A"] = build_progA()
    ncA = _cache["A"]
    As1 = np.ascontiguousarray(np.concatenate([A1[:F], A1[F:]], axis=1))
    Wf1 = np.concatenate([W1, W1 @ As1], axis=1).astype(np.float16)
    bf1 = np.concatenate(
        [bW1.reshape(F, 1), As1.T @ bW1.reshape(F, 1)], axis=0
    ).astype(np.float32)
    inA = []
    for c in cores:
        xT = np.ascontiguousarray(x[c * DPC : (c + 1) * DPC].T.astype(np.float16))
        inA.append({"xT": xT, "Wf": Wf1, "bf": bf1})
    resA = bass_utils.run_bass_kernel_spmd(ncA, inA, core_ids=cores)
    last_results.append(resA)
    wh = np.concatenate(
        [resA.results[c]["outT"][:F].T.astype(np.float32) for c in cores], axis=0
    )
    s_all = np.concatenate(
        [resA.results[c]["outT"][F : F + 2].astype(np.float32) for c in cores], axis=1
    )
    si_full, sj_full = s_all[0], s_all[1]

    key = ("B", sched.n_tiles, sched.w_total, tuple(sched.calls))
    if key not in _cache:
        _cache[key] = build_progB(sched)
    ncB = _cache[key]

    def launch_B(wh_full, si_f, sj_f, bA, Wn, bWn, An):
        wh16 = wh_full.astype(np.float16)
        Wfn = np.concatenate([Wn, Wn @ An], axis=1).astype(np.float16)
        bfn = np.concatenate(
            [bWn.reshape(F, 1), An.T @ bWn.reshape(F, 1)], axis=0
        ).astype(np.float32)
        inB = []
        for c in cores:
            perm = sched.perms[c]
            real = perm >= 0
            gids = c * DPC + perm[real]
            tmp = np.zeros(sched.n_tiles * 128, np.float32)
            tmp[real] = si_f[gids]
            si_arr = tmp.reshape(sched.n_tiles, 128).T
            esrc = sched.esrc[c]
            whe = wh16[esrc.ravel()].reshape(128, sched.w_total * F)
            sje = np.where(
                sched.emask[c] < 0.0, np.float32(NEG_BIG), sj_f[esrc]
            ).astype(np.float32)
            inB.append(
                {
                    "whe": whe,
                    "sje": sje,
                    "si": np.ascontiguousarray(si_arr),
                    "bA": np.full((128, 1), bA.reshape(-1)[0], np.float32),
                    "flags": sched.flags[c],
                    "Wf": Wfn,
                    "bf": bfn,
                }
            )
        res = bass_utils.run_bass_kernel_spmd(ncB, inB, core_ids=cores)
        last_results.append(res)
        whn = np.zeros((N_NODES, F), np.float32)
        sn_i = np.zeros(N_NODES, np.float32)
        sn_j = np.zeros(N_NODES, np.float32)
        for c in cores:
            perm = sched.perms[c]
            real = perm >= 0
            gids = c * DPC + perm[real]
            outT = res.results[c]["outT"].astype(np.float32)
            whn[gids] = outT[:F].T[real]
            sn_i[gids] = outT[F][real]
            sn_j[gids] = outT[F + 1][real]
        return whn, sn_i, sn_j

    As2 = np.ascontiguousarray(np.concatenate([A2[:F], A2[F:]], axis=1))
    wh2, si2, sj2 = launch_B(wh, si_full, sj_full, bA1, W2, bW2, As2)
    out, _, _ = launch_B(wh2, si2, sj2, bA2, Wfc, bfc, np.zeros((F, 2), np.float32))
    return out.astype(np.float32)


# revision 38
# speedup vs baseline: 1.0181x; 1.0181x over previous
"""GAT (2-layer) on 8 NeuronCores — Bass/Tile kernel.

Strategy (dst-sharded graph parallel):
  - Each core owns 12500 destination nodes, degree-sorted and tiled into
    128-dst tiles on a dense slot grid (slot width = exact per-tile max
    degree over all cores); adjacent similar-degree tiles are merged into
    <=128-column calls (Dmax/Dmin <= 1.25 slack).
  - Host pre-pass is index-only: the slot grid, per-slot source-node ids
    (esrc), pad masks, permutations.
  - Launch A: per-core [Wh1^T; s_i; s_j] = ([W1 | W1@A1]^T x^T + bias) in a
    single fused fp16 matmul per 512-column chunk (f32 accumulate).
  - Between launches the host stages per-core inputs by pure indexing of
    device-computed values: slot-expanded fp16 features whe = Wh16[esrc] and
    pad-masked source scalars sje, so the device streams edge data with
    large sequential DMAs at full HBM bandwidth instead of per-edge gather
    descriptors.
  - Launch B (x2, one per GAT layer), phase 1 computes every attention
    weight up front: e = leaky(s_i + s_j + bA) on DVE+ACT over the whole
    slot grid, ex = exp(e) in fp16 (scores are O(5), no max-subtract
    needed), per-call denominator reduce, reciprocal x zero-degree flags.
    Phase 2 streams calls through a software pipeline: slot-chunk DMA ->
    message multiply split ~50/50 between the gpsimd and vector engines ->
    fp16 pairwise-tree slot reduction (2x DVE mode) -> per-tile scalar-engine
    Prelu fusing the softmax normalization (scale=1/den) with the leaky
    relu -> fp16 PE transpose -> fused epilogue matmul [Wn | Wn@An] giving
    the next layer's [Wh^T; s_i; s_j] (or the final fc output for layer 2).
"""

import dataclasses
import numpy as np

import concourse.bacc as bacc
import concourse.tile as tile
from concourse import bass, mybir, bass_utils
from concourse.masks import make_identity

F32 = mybir.dt.float32
F16 = mybir.dt.float16

N_NODES = 100000
N_CORES = 8
DPC = N_NODES // N_CORES
F = 64
IN_C = 128
NSUB = 1
CALL_W = 128  # max slot-columns per vector-op call
MERGE_SLACK = 1.25  # max Dmax/Dmin when merging tiles into one call
POOL_MULT_SHARE = 0.5  # fraction of message-multiply elems on gpsimd
POOL_TAPER = 0.1  # extra gpsimd share on the last POOL_TAPER_N calls
POOL_TAPER_N = 4
NEG_BIG = -1.0e30
EPS = 1e-16
ALPHA = 0.2


@dataclasses.dataclass
class Schedule:
    n_tiles: int
    w_total: int
    calls: list  # (sub, t0, ntc, D, col0)
    tilecol: np.ndarray  # int32 [w_total]: tile index of each slot column
    perms: list  # per core: int64 [n_tiles*128], local dst or -1
    esrc: list  # per core: int32 [128, w_total] global source id per slot (0 pad)
    emask: list  # per core: f32 [128, w_total] (0 real / NEG_BIG pad)
    flags: list  # per core: f32 [128, n_tiles]


def build_schedule(edge_index: np.ndarray) -> Schedule:
    src = np.asarray(edge_index[0], dtype=np.int64)
    dst = np.asarray(edge_index[1], dtype=np.int64)
    order = np.argsort(dst, kind="stable")
    src_s = src[order]
    deg_all = np.bincount(dst, minlength=N_NODES)
    starts_all = np.concatenate([[0], np.cumsum(deg_all)])

    # per-core sub-shard dst lists (round-robin over degree-sorted order)
    core_subs = []  # [core][sub] -> local dst ids
    for c in range(N_CORES):
        deg = deg_all[c * DPC : (c + 1) * DPC]
        rank = np.argsort(deg, kind="stable")
        core_subs.append([rank[s::NSUB] for s in range(NSUB)])

    # shared tile plan: per (sub, tile): D = max over cores of tile max-deg
    tiles = []
    for s in range(NSUB):
        nt = -(-max(len(core_subs[c][s]) for c in range(N_CORES)) // 128)
        for t in range(nt):
            mx = 1
            for c in range(N_CORES):
                lst = core_subs[c][s][t * 128 : (t + 1) * 128]
                if len(lst):
                    deg = deg_all[c * DPC + lst]
                    mx = max(mx, int(deg.max()))
            assert mx <= 512, mx  # a tile above CALL_W just becomes its own call
            tiles.append((s, mx))
    n_tiles = len(tiles)

    # call plan: merge consecutive same-sub tiles with similar D
    calls = []
    i = 0
    col = 0
    while i < n_tiles:
        s, D = tiles[i]
        Dmax = Dmin = D
        ntc = 1
        while i + ntc < n_tiles:
            s2, D2 = tiles[i + ntc]
            if s2 != s:
                break
            nD, mD = max(Dmax, D2), min(Dmin, D2)
            if (ntc + 1) * nD > CALL_W or nD > MERGE_SLACK * mD:
                break
            Dmax, Dmin = nD, mD
            ntc += 1
        calls.append((s, i, ntc, Dmax, col))
        col += ntc * Dmax
        i += ntc
    w_total = col
    tilecol = np.zeros(w_total, np.int32)
    for (s_, t0, ntc, D, col0) in calls:
        for tl in range(ntc):
            tilecol[col0 + tl * D : col0 + (tl + 1) * D] = t0 + tl

    perms, esrcs, emasks, flagss = [], [], [], []
    for c in range(N_CORES):
        perm = np.full(n_tiles * 128, -1, dtype=np.int64)
        ti = 0
        for s in range(NSUB):
            nt = sum(1 for (ss, _) in tiles if ss == s)
            lst = core_subs[c][s]
            block = np.full(nt * 128, -1, dtype=np.int64)
            block[: len(lst)] = lst
            perm[ti * 128 : (ti + nt) * 128] = block
            ti += nt

        esrc = np.zeros((128, w_total), np.int32)
        emask = np.full((128, w_total), np.float32(NEG_BIG))
        for (s, t0, ntc, D, col0) in calls:
            for tl in range(ntc):
                tglob = t0 + tl
                dsts = perm[tglob * 128 : (tglob + 1) * 128]
                for p in range(128):
                    d = dsts[p]
                    if d < 0:
                        continue
                    g = c * DPC + d
                    e0, ne = starts_all[g], deg_all[g]
                    c0 = col0 + tl * D
                    esrc[p, c0 : c0 + ne] = src_s[e0 : e0 + ne]
                    emask[p, c0 : c0 + ne] = 0.0
        pflat = perm.copy()
        okdeg = (pflat >= 0) & (deg_all[np.clip(c * DPC + pflat, 0, N_NODES - 1)] > 0)
        flags = np.ascontiguousarray(
            okdeg.reshape(n_tiles, 128).T.astype(np.float32)
        )
        perms.append(perm)
        esrcs.append(esrc)
        emasks.append(emask)
        flagss.append(flags)

    return Schedule(n_tiles, w_total, calls, tilecol, perms, esrcs, emasks, flagss)


# ---------------------------------------------------------------- prog A
def build_progA(n_loc=DPC, in_c=IN_C, f=F):
    nc = bacc.Bacc("TRN2", target_bir_lowering=False, debug=False, num_devices=N_CORES)
    xT = nc.dram_tensor("xT", [in_c, n_loc], F16, kind="ExternalInput").ap()
    Wf = nc.dram_tensor("Wf", [in_c, f + 2], F16, kind="ExternalInput").ap()
    bf = nc.dram_tensor("bf", [f + 2, 1], F32, kind="ExternalInput").ap()
    outT = nc.dram_tensor("outT", [f + 2, n_loc], F16, kind="ExternalOutput").ap()

    AF = mybir.ActivationFunctionType
    OP = mybir.AluOpType

    with tile.TileContext(nc) as tc:
        with tc.tile_pool(name="sb", bufs=1) as pool, tc.tile_pool(
            name="ps", bufs=4, space="PSUM"
        ) as pps, tc.tile_pool(name="sb2", bufs=4) as pool2:
            xT_sb = pool.tile([in_c, n_loc], F16)
            Wf_sb = pool.tile([in_c, f + 2], F16)
            nc.sync.dma_start(out=Wf_sb[:], in_=Wf[:, :])
            bf_sb = pool.tile([f + 2, 1], F32)
            nc.sync.dma_start(out=bf_sb[:], in_=bf[:, :])
            NSPL = 12
            spl = -(-n_loc // NSPL)
            for k in range(NSPL):
                a, b = k * spl, min(n_loc, (k + 1) * spl)
                nc.sync.dma_start(out=xT_sb[:, a:b], in_=xT[:, a:b])

            CH = 512
            GB = 6  # chunks per output DMA
            ob = None
            nch = -(-n_loc // CH)
            for ci, c0 in enumerate(range(0, n_loc, CH)):
                ch = min(CH, n_loc - c0)
                ps_w = pps.tile([f + 2, CH], F32, space="PSUM")
                nc.tensor.matmul(
                    out=ps_w[:, :ch],
                    lhsT=Wf_sb[:],
                    rhs=xT_sb[:, c0 : c0 + ch],
                    start=True,
                    stop=True,
                )
                g = ci % GB
                if g == 0:
                    ob = pool2.tile([f + 2, GB * CH], F16, tag="ob")
                if ci % 2 == 0:
                    nc.scalar.activation(
                        out=ob[:, g * CH : g * CH + ch],
                        in_=ps_w[:, :ch],
                        func=AF.Identity,
                        bias=bf_sb[:],
                    )
                else:
                    nc.vector.tensor_scalar(
                        out=ob[:, g * CH : g * CH + ch],
                        in0=ps_w[:, :ch],
                        scalar1=bf_sb[:],
                        scalar2=None,
                        op0=OP.add,
                    )
                if g == GB - 1 or ci == nch - 1:
                    b0 = (ci - g) * CH
                    nc.sync.dma_start(
                        out=outT[:, b0 : c0 + ch], in_=ob[:, : g * CH + ch]
                    )
    nc.compile()
    return nc


# ---------------------------------------------------------------- prog B
def build_progB(sched: Schedule, f=F):
    NT = sched.n_tiles
    WTOT = sched.w_total
    nc = bacc.Bacc(
        "TRN2",
        target_bir_lowering=False,
        debug=False,
        num_devices=N_CORES,
    )
    whe_d = nc.dram_tensor("whe", [128, WTOT * f], F16, kind="ExternalInput").ap()
    sje_d = nc.dram_tensor("sje", [128, WTOT], F32, kind="ExternalInput").ap()
    si_d = nc.dram_tensor("si", [128, NT], F32, kind="ExternalInput").ap()
    bA_d = nc.dram_tensor("bA", [128, 1], F32, kind="ExternalInput").ap()
    flags_d = nc.dram_tensor("flags", [128, NT], F32, kind="ExternalInput").ap()
    Wf_d = nc.dram_tensor("Wf", [f, f + 2], F16, kind="ExternalInput").ap()
    bf_d = nc.dram_tensor("bf", [f + 2, 1], F32, kind="ExternalInput").ap()
    outT_d = nc.dram_tensor("outT", [f + 2, NT * 128], F16, kind="ExternalOutput").ap()

    X = mybir.AxisListType.X
    AF = mybir.ActivationFunctionType
    OP = mybir.AluOpType

    def v(ap, dims, off=0):
        return dataclasses.replace(
            ap,
            ap=[list(ap.ap[0])] + [list(d) for d in dims],
            offset=ap.offset + off,
        )

    with tile.TileContext(nc) as tc:
        with tc.tile_pool(name="const", bufs=1) as pc, tc.tile_pool(
            name="io", bufs=4
        ) as pio, tc.tile_pool(name="work", bufs=3) as pw, tc.tile_pool(
            name="ps", bufs=2, space="PSUM"
        ) as pps, tc.tile_pool(name="ep", bufs=2) as pep:
            flags_sb = pc.tile([128, NT], F32)
            nc.sync.dma_start(out=flags_sb[:], in_=flags_d[:, :])
            sje_sb = pc.tile([128, WTOT], F32)
            nc.sync.dma_start(out=sje_sb[:], in_=sje_d[:, :])
            si_sb = pc.tile([128, NT], F32)
            nc.sync.dma_start(out=si_sb[:], in_=si_d[:, :])
            bA_sb = pc.tile([128, 1], F32)
            nc.sync.dma_start(out=bA_sb[:], in_=bA_d[:, :])
            Wf_sb = pc.tile([f, f + 2], F16)
            nc.sync.dma_start(out=Wf_sb[:], in_=Wf_d[:, :])
            bf_sb = pc.tile([f + 2, 1], F32)
            nc.sync.dma_start(out=bf_sb[:], in_=bf_d[:, :])
            ident = pc.tile([128, 128], F16)
            make_identity(nc, ident[:])

            # ---- phase 1: attention weights for the whole slot grid ----
            # e = leaky(si + sj + bA) (sj pre-masked to -inf on pad slots);
            # ex = exp(e)  (no max-subtract: scores are O(10) so exp fits
            # fp16/f32 comfortably)
            ep_sb = pc.tile([128, WTOT], F32)
            for (s, t0, ntc, D, col0) in sched.calls:
                nc.vector.tensor_tensor(
                    out=v(ep_sb[:], [(D, ntc), (1, D)], off=col0),
                    in0=v(sje_sb[:], [(D, ntc), (1, D)], off=col0),
                    in1=si_sb[:, t0 : t0 + ntc].to_broadcast([128, ntc, D]),
                    op=OP.add,
                )
            ex16 = pc.tile([128, WTOT], F16)
            nc.scalar.activation(
                out=ep_sb[:], in_=ep_sb[:], func=AF.Prelu, alpha=ALPHA, bias=bA_sb[:]
            )
            nc.scalar.activation(out=ex16[:], in_=ep_sb[:], func=AF.Exp)
            den = pc.tile([128, NT], F32)
            for (s, t0, ntc, D, col0) in sched.calls:
                nc.vector.tensor_reduce(
                    out=den[:, t0 : t0 + ntc],
                    in_=v(ex16[:], [(D, ntc), (1, D)], off=col0),
                    axis=X,
                    op=OP.add,
                )
            rden = pc.tile([128, NT], F32)
            nc.vector.tensor_scalar(
                out=den[:], in0=den[:], scalar1=EPS, scalar2=None, op0=OP.add
            )
            nc.vector.reciprocal(out=rden[:], in_=den[:])
            nc.vector.tensor_tensor(
                out=rden[:], in0=rden[:], in1=flags_sb[:], op=OP.mult
            )

            # ---- phase 2: stream messages, reduce, epilogue ----
            CHT = 4  # tiles per epilogue chunk (512 dsts)

            def flush_chunk(ts, ntl, hTL, ob):
                cols = ntl * 128
                ps_w = pps.tile([f + 2, CHT * 128], F32, tag="psw", space="PSUM")
                nc.tensor.matmul(
                    out=ps_w[:, :cols],
                    lhsT=Wf_sb[:],
                    rhs=hTL[:, :cols],
                    start=True,
                    stop=True,
                )
                nc.scalar.activation(
                    out=ob[:, :cols],
                    in_=ps_w[:, :cols],
                    func=AF.Identity,
                    bias=bf_sb[:],
                )
                nc.sync.dma_start(
                    out=outT_d[:, ts * 128 : ts * 128 + cols],
                    in_=ob[:, :cols],
                )

            def stage1(ci):
                (s, t0, ntc, D, col0) = sched.calls[ci]
                W = ntc * D
                whe = pio.tile([128, W * f], F16, tag="whe")
                sh = POOL_MULT_SHARE + (
                    POOL_TAPER if ci >= len(sched.calls) - POOL_TAPER_N else 0.0
                )
                cp = max(1, min(W - 1, int(round(sh * W))))
                nc.sync.dma_start(
                    out=whe[:], in_=whe_d[:, col0 * f : (col0 + W) * f]
                )
                # weighted message (fp16): msg = Wh16 * ex16; each call's
                # multiply is split between gpsimd and DVE at the balance point
                msg16 = pw.tile([128, W * f], F16, tag="msg16")
                nc.gpsimd.tensor_tensor(
                    out=v(msg16[:], [(f, cp), (1, f)]),
                    in0=v(whe[:], [(f, cp), (1, f)]),
                    in1=ex16[:, col0 : col0 + cp].to_broadcast([128, cp, f]),
                    op=OP.mult,
                )
                nc.vector.tensor_tensor(
                    out=v(msg16[:], [(f, W - cp), (1, f)], off=cp * f),
                    in0=v(whe[:], [(f, W - cp), (1, f)], off=cp * f),
                    in1=ex16[:, col0 + cp : col0 + W].to_broadcast(
                        [128, W - cp, f]
                    ),
                    op=OP.mult,
                )
                return msg16

            def stage2(ci, msg16):
                (s, t0, ntc, D, col0) = sched.calls[ci]
                W = ntc * D
                # pairwise-tree reduce over the D slots (fp16 packed -> 2x DVE)
                hraw = pw.tile([128, ntc * f], F32, tag="hraw")
                if D == 1:
                    nc.vector.tensor_copy(
                        out=v(hraw[:], [(f, ntc), (1, f)]),
                        in_=v(msg16[:], [(D * f, ntc), (1, f)]),
                    )
                else:
                    p2 = 1
                    while p2 * 2 <= D:
                        p2 *= 2
                    if D > p2:
                        r = D - p2
                        nc.vector.tensor_tensor(
                            out=v(msg16[:], [(D * f, ntc), (1, r * f)]),
                            in0=v(msg16[:], [(D * f, ntc), (1, r * f)]),
                            in1=v(msg16[:], [(D * f, ntc), (1, r * f)], off=p2 * f),
                            op=OP.add,
                        )
                    while p2 > 2:
                        h = p2 // 2
                        nc.vector.tensor_tensor(
                            out=v(msg16[:], [(D * f, ntc), (1, h * f)]),
                            in0=v(msg16[:], [(D * f, ntc), (1, h * f)]),
                            in1=v(msg16[:], [(D * f, ntc), (1, h * f)], off=h * f),
                            op=OP.add,
                        )
                        p2 = h
                    nc.vector.tensor_tensor(
                        out=v(hraw[:], [(f, ntc), (1, f)]),
                        in0=v(msg16[:], [(D * f, ntc), (1, f)]),
                        in1=v(msg16[:], [(D * f, ntc), (1, f)], off=f),
                        op=OP.add,
                    )
                # h = leaky(hraw * rden): per-tile ACT fuses the softmax
                # normalization (scale) with the leaky relu
                for tl in range(ntc):
                    t = t0 + tl
                    hl = pw.tile([128, f], F16, tag="hl")
                    nc.scalar.activation(
                        out=hl[:],
                        in_=hraw[:, tl * f : (tl + 1) * f],
                        func=AF.Prelu,
                        alpha=ALPHA,
                        scale=rden[:, t : t + 1],
                    )
                    ps_t = pps.tile([f, 128], F16, tag="pst", space="PSUM")
                    nc.tensor.transpose(
                        out=ps_t[:], in_=hl[:], identity=ident[:]
                    )
                    j = tl % CHT
                    if j == 0:
                        hTL = pep.tile([f, CHT * 128], F16, tag="hTL")
                        ob = pep.tile([f + 2, CHT * 128], F16, tag="ob")
                    nc.scalar.activation(
                        out=hTL[:, j * 128 : (j + 1) * 128],
                        in_=ps_t[:],
                        func=AF.Identity,
                    )
                    if j == CHT - 1 or tl == ntc - 1:
                        flush_chunk(t - j, j + 1, hTL, ob)

            # 2-stage software pipeline: issue call ci+1's DMA+multiplies
            # before call ci's reduce/epilogue so the in-order DVE queue
            # never blocks the next multiply behind a Pool-gated reduce
            ncalls = len(sched.calls)
            prev = None
            for ci in range(ncalls):
                cur = stage1(ci)
                if prev is not None:
                    stage2(ci - 1, prev)
                prev = cur
            stage2(ncalls - 1, prev)
    nc.compile()
    return nc


# ---------------------------------------------------------------- driver
_cache = {}
last_results = []  # BassKernelResults per launch (for test.py profiling)


def kernel(x, edge_index, W1, bW1, A1, bA1, W2, bW2, A2, bA2, Wfc, bfc):
    x = np.asarray(x, dtype=np.float32)
    edge_index = np.asarray(edge_index)
    W1 = np.asarray(W1, np.float32)
    bW1 = np.asarray(bW1, np.float32)
    A1 = np.asarray(A1, np.float32)
    bA1 = np.asarray(bA1, np.float32)
    W2 = np.asarray(W2, np.float32)
    bW2 = np.asarray(bW2, np.float32)
    A2 = np.asarray(A2, np.float32)
    bA2 = np.asarray(bA2, np.float32)
    Wfc = np.asarray(Wfc, np.float32)
    bfc = np.asarray(bfc, np.float32)

    sched = build_schedule(edge_index)
    cores = list(range(N_CORES))
    last_results.clear()

    if "A" not in _cache:
        _cache["A"] = build_progA()
    ncA = _cache["A"]
    As1 = np.ascontiguousarray(np.concatenate([A1[:F], A1[F:]], axis=1))
    Wf1 = np.concatenate([W1, W1 @ As1], axis=1).astype(np.float16)
    bf1 = np.concatenate(
        [bW1.reshape(F, 1), As1.T @ bW1.reshape(F, 1)], axis=0
    ).astype(np.float32)
    inA = []
    for c in cores:
        xT = np.ascontiguousarray(x[c * DPC : (c + 1) * DPC].T.astype(np.float16))
        inA.append({"xT": xT, "Wf": Wf1, "bf": bf1})
    resA = bass_utils.run_bass_kernel_spmd(ncA, inA, core_ids=cores)
    last_results.append(resA)
    wh = np.concatenate(
        [resA.results[c]["outT"][:F].T.astype(np.float32) for c in cores], axis=0
    )
    s_all = np.concatenate(
        [resA.results[c]["outT"][F : F + 2].astype(np.float32) for c in cores], axis=1
    )
    si_full, sj_full = s_all[0], s_all[1]

    key = ("B", sched.n_tiles, sched.w_total, tuple(sched.calls))
    if key not in _cache:
        _cache[key] = build_progB(sched)
    ncB = _cache[key]

    def launch_B(wh_full, si_f, sj_f, bA, Wn, bWn, An):
        wh16 = wh_full.astype(np.float16)
        Wfn = np.concatenate([Wn, Wn @ An], axis=1).astype(np.float16)
        bfn = np.concatenate(
            [bWn.reshape(F, 1), An.T @ bWn.reshape(F, 1)], axis=0
        ).astype(np.float32)
        inB = []
        for c in cores:
            perm = sched.perms[c]
            real = perm >= 0
            gids = c * DPC + perm[real]
            tmp = np.zeros(sched.n_tiles * 128, np.float32)
            tmp[real] = si_f[gids]
            si_arr = tmp.reshape(sched.n_tiles, 128).T
            esrc = sched.esrc[c]
            whe = wh16[esrc.ravel()].reshape(128, sched.w_total * F)
            sje = np.where(
                sched.emask[c] < 0.0, np.float32(NEG_BIG), sj_f[esrc]
            ).astype(np.float32)
            inB.append(
                {
                    "whe": whe,
                    "sje": sje,
                    "si": np.ascontiguousarray(si_arr),
                    "bA": np.full((128, 1), bA.reshape(-1)[0], np.float32),
                    "flags": sched.flags[c],
                    "Wf": Wfn,
                    "bf": bfn,
                }
            )
        res = bass_utils.run_bass_kernel_spmd(ncB, inB, core_ids=cores)
        last_results.append(res)
        whn = np.zeros((N_NODES, F), np.float32)
        sn_i = np.zeros(N_NODES, np.float32)
        sn_j = np.zeros(N_NODES, np.float32)
        for c in cores:
            perm = sched.perms[c]
            real = perm >= 0
            gids = c * DPC + perm[real]
            outT = res.results[c]["outT"].astype(np.float32)
            whn[gids] = outT[:F].T[real]
            sn_i[gids] = outT[F][real]
            sn_j[gids] = outT[F + 1][real]
        return whn, sn_i, sn_j

    As2 = np.ascontiguousarray(np.concatenate([A2[:F], A2[F:]], axis=1))
    wh2, si2, sj2 = launch_B(wh, si_full, sj_full, bA1, W2, bW2, As2)
    out, _, _ = launch_B(wh2, si2, sj2, bA2, Wfc, bfc, np.zeros((F, 2), np.float32))
    return out.astype(np.float32)


# revision 39
# speedup vs baseline: 1.0191x; 1.0010x over previous
"""GAT (2-layer) on 8 NeuronCores — Bass/Tile kernel.

Strategy (dst-sharded graph parallel):
  - Each core owns 12500 destination nodes, degree-sorted and tiled into
    128-dst tiles on a dense slot grid (slot width = exact per-tile max
    degree over all cores); adjacent similar-degree tiles are merged into
    <=128-column calls (Dmax/Dmin <= 1.25 slack).
  - Host pre-pass is index-only: the slot grid, per-slot source-node ids
    (esrc), pad masks, permutations.
  - Launch A: per-core [Wh1^T; s_i; s_j] = ([W1 | W1@A1]^T x^T + bias) in a
    single fused fp16 matmul per 512-column chunk (f32 accumulate).
  - Between launches the host stages per-core inputs by pure indexing of
    device-computed values: slot-expanded fp16 features whe = Wh16[esrc] and
    pad-masked source scalars sje, so the device streams edge data with
    large sequential DMAs at full HBM bandwidth instead of per-edge gather
    descriptors.
  - Launch B (x2, one per GAT layer), phase 1 computes every attention
    weight up front: e = leaky(s_i + s_j + bA) on DVE+ACT over the whole
    slot grid, ex = exp(e) in fp16 (scores are O(5), no max-subtract
    needed), per-call denominator reduce, reciprocal x zero-degree flags.
    Phase 2 streams calls through a software pipeline: slot-chunk DMA ->
    message multiply split ~50/50 between the gpsimd and vector engines ->
    fp16 pairwise-tree slot reduction (2x DVE mode) -> per-tile scalar-engine
    Prelu fusing the softmax normalization (scale=1/den) with the leaky
    relu -> fp16 PE transpose -> fused epilogue matmul [Wn | Wn@An] giving
    the next layer's [Wh^T; s_i; s_j] (or the final fc output for layer 2).
"""

import dataclasses
import numpy as np

import concourse.bacc as bacc
import concourse.tile as tile
from concourse import bass, mybir, bass_utils
from concourse.masks import make_identity

F32 = mybir.dt.float32
F16 = mybir.dt.float16

N_NODES = 100000
N_CORES = 8
DPC = N_NODES // N_CORES
F = 64
IN_C = 128
NSUB = 1
CALL_W = 128  # max slot-columns per vector-op call
MERGE_SLACK = 1.25  # max Dmax/Dmin when merging tiles into one call
POOL_MULT_SHARE = 0.5  # fraction of message-multiply elems on gpsimd
POOL_TAPER = 0.1  # extra gpsimd share on the last POOL_TAPER_N calls
POOL_TAPER_N = 4
NEG_BIG = -1.0e30
EPS = 1e-16
ALPHA = 0.2


@dataclasses.dataclass
class Schedule:
    n_tiles: int
    w_total: int
    calls: list  # (sub, t0, ntc, D, col0)
    tilecol: np.ndarray  # int32 [w_total]: tile index of each slot column
    perms: list  # per core: int64 [n_tiles*128], local dst or -1
    esrc: list  # per core: int32 [128, w_total] global source id per slot (0 pad)
    emask: list  # per core: f32 [128, w_total] (0 real / NEG_BIG pad)
    flags: list  # per core: f32 [128, n_tiles]


def build_schedule(edge_index: np.ndarray) -> Schedule:
    src = np.asarray(edge_index[0], dtype=np.int64)
    dst = np.asarray(edge_index[1], dtype=np.int64)
    order = np.argsort(dst, kind="stable")
    src_s = src[order]
    deg_all = np.bincount(dst, minlength=N_NODES)
    starts_all = np.concatenate([[0], np.cumsum(deg_all)])

    # per-core sub-shard dst lists (round-robin over degree-sorted order)
    core_subs = []  # [core][sub] -> local dst ids
    for c in range(N_CORES):
        deg = deg_all[c * DPC : (c + 1) * DPC]
        rank = np.argsort(deg, kind="stable")
        core_subs.append([rank[s::NSUB] for s in range(NSUB)])

    # shared tile plan: per (sub, tile): D = max over cores of tile max-deg
    tiles = []
    for s in range(NSUB):
        nt = -(-max(len(core_subs[c][s]) for c in range(N_CORES)) // 128)
        for t in range(nt):
            mx = 1
            for c in range(N_CORES):
                lst = core_subs[c][s][t * 128 : (t + 1) * 128]
                if len(lst):
                    deg = deg_all[c * DPC + lst]
                    mx = max(mx, int(deg.max()))
            assert mx <= 512, mx  # a tile above CALL_W just becomes its own call
            tiles.append((s, mx))
    n_tiles = len(tiles)

    # call plan: merge consecutive same-sub tiles with similar D
    calls = []
    i = 0
    col = 0
    while i < n_tiles:
        s, D = tiles[i]
        Dmax = Dmin = D
        ntc = 1
        while i + ntc < n_tiles:
            s2, D2 = tiles[i + ntc]
            if s2 != s:
                break
            nD, mD = max(Dmax, D2), min(Dmin, D2)
            if (ntc + 1) * nD > CALL_W or nD > MERGE_SLACK * mD:
                break
            Dmax, Dmin = nD, mD
            ntc += 1
        calls.append((s, i, ntc, Dmax, col))
        col += ntc * Dmax
        i += ntc
    w_total = col
    tilecol = np.zeros(w_total, np.int32)
    for (s_, t0, ntc, D, col0) in calls:
        for tl in range(ntc):
            tilecol[col0 + tl * D : col0 + (tl + 1) * D] = t0 + tl

    perms, esrcs, emasks, flagss = [], [], [], []
    for c in range(N_CORES):
        perm = np.full(n_tiles * 128, -1, dtype=np.int64)
        ti = 0
        for s in range(NSUB):
            nt = sum(1 for (ss, _) in tiles if ss == s)
            lst = core_subs[c][s]
            block = np.full(nt * 128, -1, dtype=np.int64)
            block[: len(lst)] = lst
            perm[ti * 128 : (ti + nt) * 128] = block
            ti += nt

        esrc = np.zeros((128, w_total), np.int32)
        emask = np.full((128, w_total), np.float32(NEG_BIG))
        for (s, t0, ntc, D, col0) in calls:
            for tl in range(ntc):
                tglob = t0 + tl
                dsts = perm[tglob * 128 : (tglob + 1) * 128]
                for p in range(128):
                    d = dsts[p]
                    if d < 0:
                        continue
                    g = c * DPC + d
                    e0, ne = starts_all[g], deg_all[g]
                    c0 = col0 + tl * D
                    esrc[p, c0 : c0 + ne] = src_s[e0 : e0 + ne]
                    emask[p, c0 : c0 + ne] = 0.0
        pflat = perm.copy()
        okdeg = (pflat >= 0) & (deg_all[np.clip(c * DPC + pflat, 0, N_NODES - 1)] > 0)
        flags = np.ascontiguousarray(
            okdeg.reshape(n_tiles, 128).T.astype(np.float32)
        )
        perms.append(perm)
        esrcs.append(esrc)
        emasks.append(emask)
        flagss.append(flags)

    return Schedule(n_tiles, w_total, calls, tilecol, perms, esrcs, emasks, flagss)


# ---------------------------------------------------------------- prog A
def build_progA(n_loc=DPC, in_c=IN_C, f=F):
    nc = bacc.Bacc("TRN2", target_bir_lowering=False, debug=False, num_devices=N_CORES)
    xT = nc.dram_tensor("xT", [in_c, n_loc], F16, kind="ExternalInput").ap()
    Wf = nc.dram_tensor("Wf", [in_c, f + 2], F16, kind="ExternalInput").ap()
    bf = nc.dram_tensor("bf", [f + 2, 1], F32, kind="ExternalInput").ap()
    outT = nc.dram_tensor("outT", [f + 2, n_loc], F16, kind="ExternalOutput").ap()

    AF = mybir.ActivationFunctionType
    OP = mybir.AluOpType

    with tile.TileContext(nc) as tc:
        with tc.tile_pool(name="sb", bufs=1) as pool, tc.tile_pool(
            name="ps", bufs=4, space="PSUM"
        ) as pps, tc.tile_pool(name="sb2", bufs=4) as pool2:
            xT_sb = pool.tile([in_c, n_loc], F16)
            Wf_sb = pool.tile([in_c, f + 2], F16)
            nc.sync.dma_start(out=Wf_sb[:], in_=Wf[:, :])
            bf_sb = pool.tile([f + 2, 1], F32)
            nc.sync.dma_start(out=bf_sb[:], in_=bf[:, :])
            NSPL = 12
            spl = -(-n_loc // NSPL)
            for k in range(NSPL):
                a, b = k * spl, min(n_loc, (k + 1) * spl)
                nc.sync.dma_start(out=xT_sb[:, a:b], in_=xT[:, a:b])

            CH = 512
            GB = 6  # chunks per output DMA
            ob = None
            nch = -(-n_loc // CH)
            for ci, c0 in enumerate(range(0, n_loc, CH)):
                ch = min(CH, n_loc - c0)
                ps_w = pps.tile([f + 2, CH], F32, space="PSUM")
                nc.tensor.matmul(
                    out=ps_w[:, :ch],
                    lhsT=Wf_sb[:],
                    rhs=xT_sb[:, c0 : c0 + ch],
                    start=True,
                    stop=True,
                )
                g = ci % GB
                if g == 0:
                    ob = pool2.tile([f + 2, GB * CH], F16, tag="ob")
                if ci % 2 == 0:
                    nc.scalar.activation(
                        out=ob[:, g * CH : g * CH + ch],
                        in_=ps_w[:, :ch],
                        func=AF.Identity,
                        bias=bf_sb[:],
                    )
                else:
                    nc.vector.tensor_scalar(
                        out=ob[:, g * CH : g * CH + ch],
                        in0=ps_w[:, :ch],
                        scalar1=bf_sb[:],
                        scalar2=None,
                        op0=OP.add,
                    )
                if g == GB - 1 or ci == nch - 1:
                    b0 = (ci - g) * CH
                    nc.sync.dma_start(
                        out=outT[:, b0 : c0 + ch], in_=ob[:, : g * CH + ch]
                    )
    nc.compile()
    return nc


# ---------------------------------------------------------------- prog B
def build_progB(sched: Schedule, f=F):
    NT = sched.n_tiles
    WTOT = sched.w_total
    nc = bacc.Bacc(
        "TRN2",
        target_bir_lowering=False,
        debug=False,
        num_devices=N_CORES,
    )
    whe_d = nc.dram_tensor("whe", [128, WTOT * f], F16, kind="ExternalInput").ap()
    sje_d = nc.dram_tensor("sje", [128, WTOT], F32, kind="ExternalInput").ap()
    si_d = nc.dram_tensor("si", [128, NT], F32, kind="ExternalInput").ap()
    bA_d = nc.dram_tensor("bA", [128, 1], F32, kind="ExternalInput").ap()
    flags_d = nc.dram_tensor("flags", [128, NT], F32, kind="ExternalInput").ap()
    Wf_d = nc.dram_tensor("Wf", [f, f + 2], F16, kind="ExternalInput").ap()
    bf_d = nc.dram_tensor("bf", [f + 2, 1], F32, kind="ExternalInput").ap()
    outT_d = nc.dram_tensor("outT", [f + 2, NT * 128], F16, kind="ExternalOutput").ap()

    X = mybir.AxisListType.X
    AF = mybir.ActivationFunctionType
    OP = mybir.AluOpType

    def v(ap, dims, off=0):
        return dataclasses.replace(
            ap,
            ap=[list(ap.ap[0])] + [list(d) for d in dims],
            offset=ap.offset + off,
        )

    with tile.TileContext(nc) as tc:
        with tc.tile_pool(name="const", bufs=1) as pc, tc.tile_pool(
            name="io", bufs=4
        ) as pio, tc.tile_pool(name="work", bufs=3) as pw, tc.tile_pool(
            name="ps", bufs=2, space="PSUM"
        ) as pps, tc.tile_pool(name="ep", bufs=2) as pep:
            flags_sb = pc.tile([128, NT], F32)
            nc.sync.dma_start(out=flags_sb[:], in_=flags_d[:, :])
            sje_sb = pc.tile([128, WTOT], F32)
            nc.sync.dma_start(out=sje_sb[:], in_=sje_d[:, :])
            si_sb = pc.tile([128, NT], F32)
            nc.sync.dma_start(out=si_sb[:], in_=si_d[:, :])
            bA_sb = pc.tile([128, 1], F32)
            nc.sync.dma_start(out=bA_sb[:], in_=bA_d[:, :])
            Wf_sb = pc.tile([f, f + 2], F16)
            nc.sync.dma_start(out=Wf_sb[:], in_=Wf_d[:, :])
            bf_sb = pc.tile([f + 2, 1], F32)
            nc.sync.dma_start(out=bf_sb[:], in_=bf_d[:, :])
            ident = pc.tile([128, 128], F16)
            make_identity(nc, ident[:])

            # ---- phase 1: attention weights for the whole slot grid ----
            # e = leaky(si + sj + bA) (sj pre-masked to -inf on pad slots);
            # ex = exp(e)  (no max-subtract: scores are O(10) so exp fits
            # fp16/f32 comfortably)
            ep_sb = pc.tile([128, WTOT], F32)
            for (s, t0, ntc, D, col0) in sched.calls:
                nc.vector.tensor_tensor(
                    out=v(ep_sb[:], [(D, ntc), (1, D)], off=col0),
                    in0=v(sje_sb[:], [(D, ntc), (1, D)], off=col0),
                    in1=si_sb[:, t0 : t0 + ntc].to_broadcast([128, ntc, D]),
                    op=OP.add,
                )
            ex16 = pc.tile([128, WTOT], F16)
            nc.scalar.activation(
                out=ep_sb[:], in_=ep_sb[:], func=AF.Prelu, alpha=ALPHA, bias=bA_sb[:]
            )
            nc.scalar.activation(out=ex16[:], in_=ep_sb[:], func=AF.Exp)
            den = pc.tile([128, NT], F32)
            for (s, t0, ntc, D, col0) in sched.calls:
                nc.vector.tensor_reduce(
                    out=den[:, t0 : t0 + ntc],
                    in_=v(ex16[:], [(D, ntc), (1, D)], off=col0),
                    axis=X,
                    op=OP.add,
                )
            rden = pc.tile([128, NT], F32)
            nc.vector.tensor_scalar(
                out=den[:], in0=den[:], scalar1=EPS, scalar2=None, op0=OP.add
            )
            nc.vector.reciprocal(out=rden[:], in_=den[:])
            nc.vector.tensor_tensor(
                out=rden[:], in0=rden[:], in1=flags_sb[:], op=OP.mult
            )

            # ---- phase 2: stream messages, reduce, epilogue ----
            CHT = 4  # tiles per epilogue chunk (512 dsts)

            def flush_chunk(ts, ntl, hTL, ob):
                cols = ntl * 128
                ps_w = pps.tile([f + 2, CHT * 128], F32, tag="psw", space="PSUM")
                nc.tensor.matmul(
                    out=ps_w[:, :cols],
                    lhsT=Wf_sb[:],
                    rhs=hTL[:, :cols],
                    start=True,
                    stop=True,
                )
                nc.scalar.activation(
                    out=ob[:, :cols],
                    in_=ps_w[:, :cols],
                    func=AF.Identity,
                    bias=bf_sb[:],
                )
                nc.sync.dma_start(
                    out=outT_d[:, ts * 128 : ts * 128 + cols],
                    in_=ob[:, :cols],
                )

            def stage1(ci):
                (s, t0, ntc, D, col0) = sched.calls[ci]
                W = ntc * D
                whe = pio.tile([128, W * f], F16, tag="whe")
                sh = POOL_MULT_SHARE + (
                    POOL_TAPER if ci >= len(sched.calls) - POOL_TAPER_N else 0.0
                )
                cp = max(1, min(W - 1, int(round(sh * W))))
                nc.sync.dma_start(
                    out=whe[:], in_=whe_d[:, col0 * f : (col0 + W) * f]
                )
                # weighted message (fp16): msg = Wh16 * ex16; each call's
                # multiply is split between gpsimd and DVE at the balance point
                msg16 = pw.tile([128, W * f], F16, tag="msg16")
                nc.gpsimd.tensor_tensor(
                    out=v(msg16[:], [(f, cp), (1, f)]),
                    in0=v(whe[:], [(f, cp), (1, f)]),
                    in1=ex16[:, col0 : col0 + cp].to_broadcast([128, cp, f]),
                    op=OP.mult,
                )
                nc.vector.tensor_tensor(
                    out=v(msg16[:], [(f, W - cp), (1, f)], off=cp * f),
                    in0=v(whe[:], [(f, W - cp), (1, f)], off=cp * f),
                    in1=ex16[:, col0 + cp : col0 + W].to_broadcast(
                        [128, W - cp, f]
                    ),
                    op=OP.mult,
                )
                return msg16

            def stage2(ci, msg16):
                (s, t0, ntc, D, col0) = sched.calls[ci]
                W = ntc * D
                # pairwise-tree reduce over the D slots (fp16 packed -> 2x DVE)
                hraw = pw.tile([128, ntc * f], F32, tag="hraw")
                if D == 1:
                    nc.vector.tensor_copy(
                        out=v(hraw[:], [(f, ntc), (1, f)]),
                        in_=v(msg16[:], [(D * f, ntc), (1, f)]),
                    )
                else:
                    p2 = 1
                    while p2 * 2 <= D:
                        p2 *= 2
                    if D > p2:
                        r = D - p2
                        nc.vector.tensor_tensor(
                            out=v(msg16[:], [(D * f, ntc), (1, r * f)]),
                            in0=v(msg16[:], [(D * f, ntc), (1, r * f)]),
                            in1=v(msg16[:], [(D * f, ntc), (1, r * f)], off=p2 * f),
                            op=OP.add,
                        )
                    while p2 > 2:
                        h = p2 // 2
                        nc.vector.tensor_tensor(
                            out=v(msg16[:], [(D * f, ntc), (1, h * f)]),
                            in0=v(msg16[:], [(D * f, ntc), (1, h * f)]),
                            in1=v(msg16[:], [(D * f, ntc), (1, h * f)], off=h * f),
                            op=OP.add,
                        )
                        p2 = h
                    nc.vector.tensor_tensor(
                        out=v(hraw[:], [(f, ntc), (1, f)]),
                        in0=v(msg16[:], [(D * f, ntc), (1, f)]),
                        in1=v(msg16[:], [(D * f, ntc), (1, f)], off=f),
                        op=OP.add,
                    )
                # h = leaky(hraw * rden): per-tile ACT fuses the softmax
                # normalization (scale) with the leaky relu; the last two
                # calls run it on the (by then idle) DVE to shorten the tail
                tail_call = ci >= len(sched.calls) - 2
                if tail_call:
                    nc.vector.tensor_tensor(
                        out=v(hraw[:], [(f, ntc), (1, f)]),
                        in0=v(hraw[:], [(f, ntc), (1, f)]),
                        in1=rden[:, t0 : t0 + ntc].to_broadcast([128, ntc, f]),
                        op=OP.mult,
                    )
                    hl_all = pw.tile([128, ntc * f], F16, tag="hl_all")
                    nc.vector.scalar_tensor_tensor(
                        out=hl_all[:],
                        in0=hraw[:],
                        scalar=ALPHA,
                        in1=hraw[:],
                        op0=OP.mult,
                        op1=OP.max,
                    )
                for tl in range(ntc):
                    t = t0 + tl
                    if tail_call:
                        hl = hl_all[:, tl * f : (tl + 1) * f]
                    else:
                        hlt = pw.tile([128, f], F16, tag="hl")
                        nc.scalar.activation(
                            out=hlt[:],
                            in_=hraw[:, tl * f : (tl + 1) * f],
                            func=AF.Prelu,
                            alpha=ALPHA,
                            scale=rden[:, t : t + 1],
                        )
                        hl = hlt[:]
                    ps_t = pps.tile([f, 128], F16, tag="pst", space="PSUM")
                    nc.tensor.transpose(
                        out=ps_t[:], in_=hl, identity=ident[:]
                    )
                    j = tl % CHT
                    if j == 0:
                        hTL = pep.tile([f, CHT * 128], F16, tag="hTL")
                        ob = pep.tile([f + 2, CHT * 128], F16, tag="ob")
                    nc.scalar.activation(
                        out=hTL[:, j * 128 : (j + 1) * 128],
                        in_=ps_t[:],
                        func=AF.Identity,
                    )
                    if j == CHT - 1 or tl == ntc - 1:
                        flush_chunk(t - j, j + 1, hTL, ob)

            # 2-stage software pipeline: issue call ci+1's DMA+multiplies
            # before call ci's reduce/epilogue so the in-order DVE queue
            # never blocks the next multiply behind a Pool-gated reduce
            ncalls = len(sched.calls)
            prev = None
            for ci in range(ncalls):
                cur = stage1(ci)
                if prev is not None:
                    stage2(ci - 1, prev)
                prev = cur
            stage2(ncalls - 1, prev)
    nc.compile()
    return nc


# ---------------------------------------------------------------- driver
_cache = {}
last_results = []  # BassKernelResults per launch (for test.py profiling)


def kernel(x, edge_index, W1, bW1, A1, bA1, W2, bW2, A2, bA2, Wfc, bfc):
    x = np.asarray(x, dtype=np.float32)
    edge_index = np.asarray(edge_index)
    W1 = np.asarray(W1, np.float32)
    bW1 = np.asarray(bW1, np.float32)
    A1 = np.asarray(A1, np.float32)
    bA1 = np.asarray(bA1, np.float32)
    W2 = np.asarray(W2, np.float32)
    bW2 = np.asarray(bW2, np.float32)
    A2 = np.asarray(A2, np.float32)
    bA2 = np.asarray(bA2, np.float32)
    Wfc = np.asarray(Wfc, np.float32)
    bfc = np.asarray(bfc, np.float32)

    sched = build_schedule(edge_index)
    cores = list(range(N_CORES))
    last_results.clear()

    if "A" not in _cache:
        _cache["A"] = build_progA()
    ncA = _cache["A"]
    As1 = np.ascontiguousarray(np.concatenate([A1[:F], A1[F:]], axis=1))
    Wf1 = np.concatenate([W1, W1 @ As1], axis=1).astype(np.float16)
    bf1 = np.concatenate(
        [bW1.reshape(F, 1), As1.T @ bW1.reshape(F, 1)], axis=0
    ).astype(np.float32)
    inA = []
    for c in cores:
        xT = np.ascontiguousarray(x[c * DPC : (c + 1) * DPC].T.astype(np.float16))
        inA.append({"xT": xT, "Wf": Wf1, "bf": bf1})
    resA = bass_utils.run_bass_kernel_spmd(ncA, inA, core_ids=cores)
    last_results.append(resA)
    wh = np.concatenate(
        [resA.results[c]["outT"][:F].T.astype(np.float32) for c in cores], axis=0
    )
    s_all = np.concatenate(
        [resA.results[c]["outT"][F : F + 2].astype(np.float32) for c in cores], axis=1
    )
    si_full, sj_full = s_all[0], s_all[1]

    key = ("B", sched.n_tiles, sched.w_total, tuple(sched.calls))
    if key not in _cache:
        _cache[key] = build_progB(sched)
    ncB = _cache[key]

    def launch_B(wh_full, si_f, sj_f, bA, Wn, bWn, An):
        wh16 = wh_full.astype(np.float16)
        Wfn = np.concatenate([Wn, Wn @ An], axis=1).astype(np.float16)
        bfn = np.concatenate(
            [bWn.reshape(F, 1), An.T @ bWn.reshape(F, 1)], axis=0
        ).astype(np.float32)
        inB = []
        for c in cores:
            perm = sched.perms[c]
            real = perm >= 0
            gids = c * DPC + perm[real]
            tmp = np.zeros(sched.n_tiles * 128, np.float32)
            tmp[real] = si_f[gids]
            si_arr = tmp.reshape(sched.n_tiles, 128).T
            esrc = sched.esrc[c]
            whe = wh16[esrc.ravel()].reshape(128, sched.w_total * F)
            sje = np.where(
                sched.emask[c] < 0.0, np.float32(NEG_BIG), sj_f[esrc]
            ).astype(np.float32)
            inB.append(
                {
                    "whe": whe,
                    "sje": sje,
                    "si": np.ascontiguousarray(si_arr),
                    "bA": np.full((128, 1), bA.reshape(-1)[0], np.float32),
                    "flags": sched.flags[c],
                    "Wf": Wfn,
                    "bf": bfn,
                }
            )
        res = bass_utils.run_bass_kernel_spmd(ncB, inB, core_ids=cores)
        last_results.append(res)
        whn = np.zeros((N_NODES, F), np.float32)
        sn_i = np.zeros(N_NODES, np.float32)
        sn_j = np.zeros(N_NODES, np.float32)
        for c in cores:
            perm = sched.perms[c]
            real = perm >= 0
            gids = c * DPC + perm[real]
            outT = res.results[c]["outT"].astype(np.float32)
            whn[gids] = outT[:F].T[real]
            sn_i[gids] = outT[F][real]
            sn_j[gids] = outT[F + 1][real]
        return whn, sn_i, sn_j

    As2 = np.ascontiguousarray(np.concatenate([A2[:F], A2[F:]], axis=1))
    wh2, si2, sj2 = launch_B(wh, si_full, sj_full, bA1, W2, bW2, As2)
    out, _, _ = launch_B(wh2, si2, sj2, bA2, Wfc, bfc, np.zeros((F, 2), np.float32))
    return out.astype(np.float32)


# revision 49
# speedup vs baseline: 1.0384x; 1.0189x over previous
"""GAT (2-layer) on 8 NeuronCores — Bass/Tile kernel.

Strategy (dst-sharded graph parallel):
  - Each core owns 12500 destination nodes, degree-sorted and tiled into
    128-dst tiles on a dense slot grid (slot width = exact per-tile max
    degree over all cores); adjacent similar-degree tiles are merged into
    <=128-column calls (Dmax/Dmin <= 1.25 slack).
  - Host pre-pass is index-only: the slot grid, per-slot source-node ids
    (esrc), pad masks, permutations.
  - Launch A: per-core [Wh1^T; s_i; s_j] = ([W1 | W1@A1]^T x^T + bias) in a
    single fused fp16 matmul per 512-column chunk (f32 accumulate).
  - Between launches the host stages per-core inputs by pure indexing of
    device-computed values: slot-expanded fp16 features whe = Wh16[esrc] and
    pad-masked source scalars sje, so the device streams edge data with
    large sequential DMAs at full HBM bandwidth instead of per-edge gather
    descriptors.
  - Launch B (x2, one per GAT layer), phase 1 computes every attention
    weight up front: e = leaky(s_i + s_j + bA) on DVE+ACT over the whole
    slot grid, ex = exp(e) in fp16 (scores are O(5), no max-subtract
    needed), per-call denominator reduce, reciprocal x zero-degree flags.
    Phase 2 streams calls through a software pipeline: slot-chunk DMA ->
    message multiply split ~50/50 between the gpsimd and vector engines ->
    fp16 pairwise-tree slot reduction (2x DVE mode) -> per-tile scalar-engine
    Prelu fusing the softmax normalization (scale=1/den) with the leaky
    relu -> fp16 PE transpose -> fused epilogue matmul [Wn | Wn@An] giving
    the next layer's [Wh^T; s_i; s_j] (or the final fc output for layer 2).
"""

import dataclasses
import numpy as np

import concourse.bacc as bacc
import concourse.tile as tile
from concourse import bass, mybir, bass_utils
from concourse.masks import make_identity

F32 = mybir.dt.float32
F16 = mybir.dt.float16

N_NODES = 100000
N_CORES = 8
DPC = N_NODES // N_CORES
F = 64
IN_C = 128
NSUB = 1
CALL_W = 128  # max slot-columns per vector-op call
MERGE_SLACK = 1.25  # max Dmax/Dmin when merging tiles into one call
POOL_MULT_SHARE = 0.5  # fraction of message-multiply elems on gpsimd
POOL_TAPER = 0.1  # extra gpsimd share on the last POOL_TAPER_N calls
POOL_TAPER_N = 4
NEG_BIG = -1.0e30
EPS = 1e-16
ALPHA = 0.2


@dataclasses.dataclass
class Schedule:
    n_tiles: int
    w_total: int
    calls: list  # (sub, t0, ntc, D, col0)
    tilecol: np.ndarray  # int32 [w_total]: tile index of each slot column
    perms: list  # per core: int64 [n_tiles*128], local dst or -1
    esrc: list  # per core: int32 [128, w_total] global source id per slot (0 pad)
    emask: list  # per core: f32 [128, w_total] (0 real / NEG_BIG pad)
    flags: list  # per core: f32 [128, n_tiles]


def build_schedule(edge_index: np.ndarray) -> Schedule:
    src = np.asarray(edge_index[0], dtype=np.int64)
    dst = np.asarray(edge_index[1], dtype=np.int64)
    order = np.argsort(dst, kind="stable")
    src_s = src[order]
    deg_all = np.bincount(dst, minlength=N_NODES)
    starts_all = np.concatenate([[0], np.cumsum(deg_all)])

    # per-core sub-shard dst lists (round-robin over degree-sorted order)
    core_subs = []  # [core][sub] -> local dst ids
    for c in range(N_CORES):
        deg = deg_all[c * DPC : (c + 1) * DPC]
        rank = np.argsort(deg, kind="stable")
        core_subs.append([rank[s::NSUB] for s in range(NSUB)])

    # shared tile plan: per (sub, tile): D = max over cores of tile max-deg
    tiles = []
    for s in range(NSUB):
        nt = -(-max(len(core_subs[c][s]) for c in range(N_CORES)) // 128)
        for t in range(nt):
            mx = 1
            for c in range(N_CORES):
                lst = core_subs[c][s][t * 128 : (t + 1) * 128]
                if len(lst):
                    deg = deg_all[c * DPC + lst]
                    mx = max(mx, int(deg.max()))
            assert mx <= 512, mx  # a tile above CALL_W just becomes its own call
            tiles.append((s, mx))
    n_tiles = len(tiles)

    # call plan: merge consecutive same-sub tiles with similar D
    calls = []
    i = 0
    col = 0
    while i < n_tiles:
        s, D = tiles[i]
        Dmax = Dmin = D
        ntc = 1
        while i + ntc < n_tiles:
            s2, D2 = tiles[i + ntc]
            if s2 != s:
                break
            nD, mD = max(Dmax, D2), min(Dmin, D2)
            if (ntc + 1) * nD > CALL_W or nD > MERGE_SLACK * mD:
                break
            Dmax, Dmin = nD, mD
            ntc += 1
        calls.append((s, i, ntc, Dmax, col))
        col += ntc * Dmax
        i += ntc
    w_total = col
    tilecol = np.zeros(w_total, np.int32)
    for (s_, t0, ntc, D, col0) in calls:
        for tl in range(ntc):
            tilecol[col0 + tl * D : col0 + (tl + 1) * D] = t0 + tl

    perms, esrcs, emasks, flagss = [], [], [], []
    for c in range(N_CORES):
        perm = np.full(n_tiles * 128, -1, dtype=np.int64)
        ti = 0
        for s in range(NSUB):
            nt = sum(1 for (ss, _) in tiles if ss == s)
            lst = core_subs[c][s]
            block = np.full(nt * 128, -1, dtype=np.int64)
            block[: len(lst)] = lst
            perm[ti * 128 : (ti + nt) * 128] = block
            ti += nt

        esrc = np.zeros((128, w_total), np.int32)
        emask = np.full((128, w_total), np.float32(NEG_BIG))
        for (s, t0, ntc, D, col0) in calls:
            for tl in range(ntc):
                tglob = t0 + tl
                dsts = perm[tglob * 128 : (tglob + 1) * 128]
                for p in range(128):
                    d = dsts[p]
                    if d < 0:
                        continue
                    g = c * DPC + d
                    e0, ne = starts_all[g], deg_all[g]
                    c0 = col0 + tl * D
                    esrc[p, c0 : c0 + ne] = src_s[e0 : e0 + ne]
                    emask[p, c0 : c0 + ne] = 0.0
        pflat = perm.copy()
        okdeg = (pflat >= 0) & (deg_all[np.clip(c * DPC + pflat, 0, N_NODES - 1)] > 0)
        flags = np.ascontiguousarray(
            okdeg.reshape(n_tiles, 128).T.astype(np.float32)
        )
        perms.append(perm)
        esrcs.append(esrc)
        emasks.append(emask)
        flagss.append(flags)

    return Schedule(n_tiles, w_total, calls, tilecol, perms, esrcs, emasks, flagss)


# ---------------------------------------------------------------- prog A
def build_progA(n_loc=DPC, in_c=IN_C, f=F):
    nc = bacc.Bacc("TRN2", target_bir_lowering=False, debug=False, num_devices=N_CORES)
    xT = nc.dram_tensor("xT", [in_c, n_loc], F16, kind="ExternalInput").ap()
    Wf = nc.dram_tensor("Wf", [in_c, f + 2], F16, kind="ExternalInput").ap()
    bf = nc.dram_tensor("bf", [f + 2, 1], F32, kind="ExternalInput").ap()
    outT = nc.dram_tensor("outT", [f + 2, n_loc], F16, kind="ExternalOutput").ap()

    AF = mybir.ActivationFunctionType
    OP = mybir.AluOpType

    with tile.TileContext(nc) as tc:
        with tc.tile_pool(name="sb", bufs=1) as pool, tc.tile_pool(
            name="ps", bufs=4, space="PSUM"
        ) as pps, tc.tile_pool(name="sb2", bufs=4) as pool2:
            xT_sb = pool.tile([in_c, n_loc], F16)
            Wf_sb = pool.tile([in_c, f + 2], F16)
            nc.sync.dma_start(out=Wf_sb[:], in_=Wf[:, :])
            bf_sb = pool.tile([f + 2, 1], F32)
            nc.sync.dma_start(out=bf_sb[:], in_=bf[:, :])
            NSPL = 12
            spl = -(-n_loc // NSPL)
            for k in range(NSPL):
                a, b = k * spl, min(n_loc, (k + 1) * spl)
                nc.sync.dma_start(out=xT_sb[:, a:b], in_=xT[:, a:b])

            CH = 512
            GB = 6  # chunks per output DMA
            ob = None
            nch = -(-n_loc // CH)
            for ci, c0 in enumerate(range(0, n_loc, CH)):
                ch = min(CH, n_loc - c0)
                ps_w = pps.tile([f + 2, CH], F32, space="PSUM")
                nc.tensor.matmul(
                    out=ps_w[:, :ch],
                    lhsT=Wf_sb[:],
                    rhs=xT_sb[:, c0 : c0 + ch],
                    start=True,
                    stop=True,
                )
                g = ci % GB
                if g == 0:
                    ob = pool2.tile([f + 2, GB * CH], F16, tag="ob")
                if ci % 2 == 0:
                    nc.scalar.activation(
                        out=ob[:, g * CH : g * CH + ch],
                        in_=ps_w[:, :ch],
                        func=AF.Identity,
                        bias=bf_sb[:],
                    )
                else:
                    nc.vector.tensor_scalar(
                        out=ob[:, g * CH : g * CH + ch],
                        in0=ps_w[:, :ch],
                        scalar1=bf_sb[:],
                        scalar2=None,
                        op0=OP.add,
                    )
                if g == GB - 1 or ci == nch - 1:
                    b0 = (ci - g) * CH
                    nc.sync.dma_start(
                        out=outT[:, b0 : c0 + ch], in_=ob[:, : g * CH + ch]
                    )
    nc.compile()
    return nc


# ---------------------------------------------------------------- prog B
def build_progB(sched: Schedule, f=F):
    NT = sched.n_tiles
    WTOT = sched.w_total
    nc = bacc.Bacc(
        "TRN2",
        target_bir_lowering=False,
        debug=False,
        num_devices=N_CORES,
    )
    whe_d = nc.dram_tensor("whe", [128, WTOT * f], F16, kind="ExternalInput").ap()
    sje_d = nc.dram_tensor("sje", [128, WTOT], F32, kind="ExternalInput").ap()
    si_d = nc.dram_tensor("si", [128, NT], F32, kind="ExternalInput").ap()
    bA_d = nc.dram_tensor("bA", [128, 1], F32, kind="ExternalInput").ap()
    flags_d = nc.dram_tensor("flags", [128, NT], F32, kind="ExternalInput").ap()
    Wf_d = nc.dram_tensor("Wf", [f, f + 2], F16, kind="ExternalInput").ap()
    bf_d = nc.dram_tensor("bf", [f + 2, 1], F32, kind="ExternalInput").ap()
    outT_d = nc.dram_tensor("outT", [f + 2, NT * 128], F16, kind="ExternalOutput").ap()

    X = mybir.AxisListType.X
    AF = mybir.ActivationFunctionType
    OP = mybir.AluOpType

    def v(ap, dims, off=0):
        return dataclasses.replace(
            ap,
            ap=[list(ap.ap[0])] + [list(d) for d in dims],
            offset=ap.offset + off,
        )

    with tile.TileContext(nc) as tc:
        with tc.tile_pool(name="const", bufs=1) as pc, tc.tile_pool(
            name="io", bufs=4
        ) as pio, tc.tile_pool(name="work", bufs=3) as pw, tc.tile_pool(
            name="ps", bufs=2, space="PSUM"
        ) as pps, tc.tile_pool(name="ep", bufs=2) as pep:
            # critical-path consts first on the SP DMA queue, then the first
            # call's whe (Pool's multiply slice first so it can start while
            # the rest streams), then the consts only needed later
            sje_sb = pc.tile([128, WTOT], F32)
            nc.sync.dma_start(out=sje_sb[:], in_=sje_d[:, :])
            si_sb = pc.tile([128, NT], F32)
            nc.sync.dma_start(out=si_sb[:], in_=si_d[:, :])
            bA_sb = pc.tile([128, 1], F32)
            nc.sync.dma_start(out=bA_sb[:], in_=bA_d[:, :])
            (s0, t00, ntc0, D0, col00) = sched.calls[0]
            W0 = ntc0 * D0
            sh0 = POOL_MULT_SHARE + (
                POOL_TAPER if len(sched.calls) <= POOL_TAPER_N else 0.0
            )
            cp0 = max(1, min(W0 - 1, int(round(sh0 * W0))))
            whe0 = pio.tile([128, W0 * f], F16, tag="whe")
            nc.sync.dma_start(
                out=whe0[:, : cp0 * f], in_=whe_d[:, col00 * f : (col00 + cp0) * f]
            )
            nc.sync.dma_start(
                out=whe0[:, cp0 * f :],
                in_=whe_d[:, (col00 + cp0) * f : (col00 + W0) * f],
            )
            flags_sb = pc.tile([128, NT], F32)
            nc.sync.dma_start(out=flags_sb[:], in_=flags_d[:, :])
            Wf_sb = pc.tile([f, f + 2], F16)
            nc.sync.dma_start(out=Wf_sb[:], in_=Wf_d[:, :])
            bf_sb = pc.tile([f + 2, 1], F32)
            nc.sync.dma_start(out=bf_sb[:], in_=bf_d[:, :])
            ident = pc.tile([128, 128], F16)
            make_identity(nc, ident[:])

            # ---- phase 1: attention weights for the whole slot grid ----
            # e = leaky(si + sj + bA) (sj pre-masked to -inf on pad slots);
            # ex = exp(e)  (no max-subtract: scores are O(10) so exp fits
            # fp16/f32 comfortably)
            ep_sb = pc.tile([128, WTOT], F32)
            for (s, t0, ntc, D, col0) in sched.calls:
                nc.vector.tensor_tensor(
                    out=v(ep_sb[:], [(D, ntc), (1, D)], off=col0),
                    in0=v(sje_sb[:], [(D, ntc), (1, D)], off=col0),
                    in1=si_sb[:, t0 : t0 + ntc].to_broadcast([128, ntc, D]),
                    op=OP.add,
                )
            ex16 = pc.tile([128, WTOT], F16)
            nc.scalar.activation(
                out=ep_sb[:], in_=ep_sb[:], func=AF.Prelu, alpha=ALPHA, bias=bA_sb[:]
            )
            nc.scalar.activation(out=ex16[:], in_=ep_sb[:], func=AF.Exp)
            den = pc.tile([128, NT], F32)
            rden = pc.tile([128, NT], F32)

            def denominators():
                for (s, t0, ntc, D, col0) in sched.calls:
                    nc.vector.tensor_reduce(
                        out=den[:, t0 : t0 + ntc],
                        in_=v(ex16[:], [(D, ntc), (1, D)], off=col0),
                        axis=X,
                        op=OP.add,
                    )
                nc.vector.tensor_scalar(
                    out=den[:], in0=den[:], scalar1=EPS, scalar2=None, op0=OP.add
                )
                nc.vector.reciprocal(out=rden[:], in_=den[:])
                nc.vector.tensor_tensor(
                    out=rden[:], in0=rden[:], in1=flags_sb[:], op=OP.mult
                )

            # ---- phase 2: stream messages, reduce, epilogue ----
            CHT = 4  # tiles per epilogue chunk (512 dsts)

            def flush_chunk(ts, ntl, hTL, ob):
                cols = ntl * 128
                ps_w = pps.tile([f + 2, CHT * 128], F32, tag="psw", space="PSUM")
                nc.tensor.matmul(
                    out=ps_w[:, :cols],
                    lhsT=Wf_sb[:],
                    rhs=hTL[:, :cols],
                    start=True,
                    stop=True,
                )
                nc.scalar.activation(
                    out=ob[:, :cols],
                    in_=ps_w[:, :cols],
                    func=AF.Identity,
                    bias=bf_sb[:],
                )
                nc.sync.dma_start(
                    out=outT_d[:, ts * 128 : ts * 128 + cols],
                    in_=ob[:, :cols],
                )

            def stage1(ci, whe=None):
                (s, t0, ntc, D, col0) = sched.calls[ci]
                W = ntc * D
                sh = POOL_MULT_SHARE + (
                    POOL_TAPER if ci >= len(sched.calls) - POOL_TAPER_N else 0.0
                )
                cp = max(1, min(W - 1, int(round(sh * W))))
                if whe is None:
                    whe = pio.tile([128, W * f], F16, tag="whe")
                    nc.sync.dma_start(
                        out=whe[:], in_=whe_d[:, col0 * f : (col0 + W) * f]
                    )
                # weighted message (fp16): msg = Wh16 * ex16; each call's
                # multiply is split between gpsimd and DVE at the balance point
                msg16 = pw.tile([128, W * f], F16, tag="msg16")
                nc.gpsimd.tensor_tensor(
                    out=v(msg16[:], [(f, cp), (1, f)]),
                    in0=v(whe[:], [(f, cp), (1, f)]),
                    in1=ex16[:, col0 : col0 + cp].to_broadcast([128, cp, f]),
                    op=OP.mult,
                )
                nc.vector.tensor_tensor(
                    out=v(msg16[:], [(f, W - cp), (1, f)], off=cp * f),
                    in0=v(whe[:], [(f, W - cp), (1, f)], off=cp * f),
                    in1=ex16[:, col0 + cp : col0 + W].to_broadcast(
                        [128, W - cp, f]
                    ),
                    op=OP.mult,
                )
                return msg16

            def stage2(ci, msg16):
                (s, t0, ntc, D, col0) = sched.calls[ci]
                W = ntc * D
                # pairwise-tree reduce over the D slots (fp16 packed -> 2x DVE)
                hraw = pw.tile([128, ntc * f], F32, tag="hraw")
                if D == 1:
                    nc.vector.tensor_copy(
                        out=v(hraw[:], [(f, ntc), (1, f)]),
                        in_=v(msg16[:], [(D * f, ntc), (1, f)]),
                    )
                else:
                    p2 = 1
                    while p2 * 2 <= D:
                        p2 *= 2
                    if D > p2:
                        r = D - p2
                        nc.vector.tensor_tensor(
                            out=v(msg16[:], [(D * f, ntc), (1, r * f)]),
                            in0=v(msg16[:], [(D * f, ntc), (1, r * f)]),
                            in1=v(msg16[:], [(D * f, ntc), (1, r * f)], off=p2 * f),
                            op=OP.add,
                        )
                    while p2 > 2:
                        h = p2 // 2
                        nc.vector.tensor_tensor(
                            out=v(msg16[:], [(D * f, ntc), (1, h * f)]),
                            in0=v(msg16[:], [(D * f, ntc), (1, h * f)]),
                            in1=v(msg16[:], [(D * f, ntc), (1, h * f)], off=h * f),
                            op=OP.add,
                        )
                        p2 = h
                    nc.vector.tensor_tensor(
                        out=v(hraw[:], [(f, ntc), (1, f)]),
                        in0=v(msg16[:], [(D * f, ntc), (1, f)]),
                        in1=v(msg16[:], [(D * f, ntc), (1, f)], off=f),
                        op=OP.add,
                    )
                # h = leaky(hraw * rden): per-tile ACT fuses the softmax
                # normalization (scale) with the leaky relu; the last two
                # calls run it on the (by then idle) DVE to shorten the tail
                tail_call = ci >= len(sched.calls) - 2
                if tail_call:
                    nc.vector.tensor_tensor(
                        out=v(hraw[:], [(f, ntc), (1, f)]),
                        in0=v(hraw[:], [(f, ntc), (1, f)]),
                        in1=rden[:, t0 : t0 + ntc].to_broadcast([128, ntc, f]),
                        op=OP.mult,
                    )
                    hl_all = pw.tile([128, ntc * f], F16, tag="hl_all")
                    nc.vector.scalar_tensor_tensor(
                        out=hl_all[:],
                        in0=hraw[:],
                        scalar=ALPHA,
                        in1=hraw[:],
                        op0=OP.mult,
                        op1=OP.max,
                    )
                for tl in range(ntc):
                    t = t0 + tl
                    if tail_call:
                        hl = hl_all[:, tl * f : (tl + 1) * f]
                    else:
                        hlt = pw.tile([128, f], F16, tag="hl")
                        nc.scalar.activation(
                            out=hlt[:],
                            in_=hraw[:, tl * f : (tl + 1) * f],
                            func=AF.Prelu,
                            alpha=ALPHA,
                            scale=rden[:, t : t + 1],
                        )
                        hl = hlt[:]
                    ps_t = pps.tile([f, 128], F16, tag="pst", space="PSUM")
                    nc.tensor.transpose(
                        out=ps_t[:], in_=hl, identity=ident[:]
                    )
                    j = tl % CHT
                    if j == 0:
                        hTL = pep.tile([f, CHT * 128], F16, tag="hTL")
                        ob = pep.tile([f + 2, CHT * 128], F16, tag="ob")
                    nc.scalar.activation(
                        out=hTL[:, j * 128 : (j + 1) * 128],
                        in_=ps_t[:],
                        func=AF.Identity,
                    )
                    if j == CHT - 1 or tl == ntc - 1:
                        flush_chunk(t - j, j + 1, hTL, ob)

            # 2-stage software pipeline: issue call ci+1's DMA+multiplies
            # before call ci's reduce/epilogue so the in-order DVE queue
            # never blocks the next multiply behind a Pool-gated reduce
            ncalls = len(sched.calls)
            prev = stage1(0, whe0)
            denominators()  # after call 0's multiply so DVE starts it sooner
            for ci in range(1, ncalls):
                cur = stage1(ci)
                stage2(ci - 1, prev)
                prev = cur
            stage2(ncalls - 1, prev)
    nc.compile()
    return nc


# ---------------------------------------------------------------- driver
_cache = {}
last_results = []  # BassKernelResults per launch (for test.py profiling)


def kernel(x, edge_index, W1, bW1, A1, bA1, W2, bW2, A2, bA2, Wfc, bfc):
    x = np.asarray(x, dtype=np.float32)
    edge_index = np.asarray(edge_index)
    W1 = np.asarray(W1, np.float32)
    bW1 = np.asarray(bW1, np.float32)
    A1 = np.asarray(A1, np.float32)
    bA1 = np.asarray(bA1, np.float32)
    W2 = np.asarray(W2, np.float32)
    bW2 = np.asarray(bW2, np.float32)
    A2 = np.asarray(A2, np.float32)
    bA2 = np.asarray(bA2, np.float32)
    Wfc = np.asarray(Wfc, np.float32)
    bfc = np.asarray(bfc, np.float32)

    sched = build_schedule(edge_index)
    cores = list(range(N_CORES))
    last_results.clear()

    if "A" not in _cache:
        _cache["A"] = build_progA()
    ncA = _cache["A"]
    As1 = np.ascontiguousarray(np.concatenate([A1[:F], A1[F:]], axis=1))
    Wf1 = np.concatenate([W1, W1 @ As1], axis=1).astype(np.float16)
    bf1 = np.concatenate(
        [bW1.reshape(F, 1), As1.T @ bW1.reshape(F, 1)], axis=0
    ).astype(np.float32)
    inA = []
    for c in cores:
        xT = np.ascontiguousarray(x[c * DPC : (c + 1) * DPC].T.astype(np.float16))
        inA.append({"xT": xT, "Wf": Wf1, "bf": bf1})
    resA = bass_utils.run_bass_kernel_spmd(ncA, inA, core_ids=cores)
    last_results.append(resA)
    wh = np.concatenate(
        [resA.results[c]["outT"][:F].T.astype(np.float32) for c in cores], axis=0
    )
    s_all = np.concatenate(
        [resA.results[c]["outT"][F : F + 2].astype(np.float32) for c in cores], axis=1
    )
    si_full, sj_full = s_all[0], s_all[1]

    key = ("B", sched.n_tiles, sched.w_total, tuple(sched.calls))
    if key not in _cache:
        _cache[key] = build_progB(sched)
    ncB = _cache[key]

    def launch_B(wh_full, si_f, sj_f, bA, Wn, bWn, An):
        wh16 = wh_full.astype(np.float16)
        Wfn = np.concatenate([Wn, Wn @ An], axis=1).astype(np.float16)
        bfn = np.concatenate(
            [bWn.reshape(F, 1), An.T @ bWn.reshape(F, 1)], axis=0
        ).astype(np.float32)
        inB = []
        for c in cores:
            perm = sched.perms[c]
            real = perm >= 0
            gids = c * DPC + perm[real]
            tmp = np.zeros(sched.n_tiles * 128, np.float32)
            tmp[real] = si_f[gids]
            si_arr = tmp.reshape(sched.n_tiles, 128).T
            esrc = sched.esrc[c]
            whe = wh16[esrc.ravel()].reshape(128, sched.w_total * F)
            sje = np.where(
                sched.emask[c] < 0.0, np.float32(NEG_BIG), sj_f[esrc]
            ).astype(np.float32)
            inB.append(
                {
                    "whe": whe,
                    "sje": sje,
                    "si": np.ascontiguousarray(si_arr),
                    "bA": np.full((128, 1), bA.reshape(-1)[0], np.float32),
                    "flags": sched.flags[c],
                    "Wf": Wfn,
                    "bf": bfn,
                }
            )
        res = bass_utils.run_bass_kernel_spmd(ncB, inB, core_ids=cores)
        last_results.append(res)
        whn = np.zeros((N_NODES, F), np.float32)
        sn_i = np.zeros(N_NODES, np.float32)
        sn_j = np.zeros(N_NODES, np.float32)
        for c in cores:
            perm = sched.perms[c]
            real = perm >= 0
            gids = c * DPC + perm[real]
            outT = res.results[c]["outT"].astype(np.float32)
            whn[gids] = outT[:F].T[real]
            sn_i[gids] = outT[F][real]
            sn_j[gids] = outT[F + 1][real]
        return whn, sn_i, sn_j

    As2 = np.ascontiguousarray(np.concatenate([A2[:F], A2[F:]], axis=1))
    wh2, si2, sj2 = launch_B(wh, si_full, sj_full, bA1, W2, bW2, As2)
    out, _, _ = launch_B(wh2, si2, sj2, bA2, Wfc, bfc, np.zeros((F, 2), np.float32))
    return out.astype(np.float32)


# revision 50
# speedup vs baseline: 1.0424x; 1.0039x over previous
"""GAT (2-layer) on 8 NeuronCores — Bass/Tile kernel.

Strategy (dst-sharded graph parallel):
  - Each core owns 12500 destination nodes, degree-sorted and tiled into
    128-dst tiles on a dense slot grid (slot width = exact per-tile max
    degree over all cores); adjacent similar-degree tiles are merged into
    <=128-column calls (Dmax/Dmin <= 1.25 slack).
  - Host pre-pass is index-only: the slot grid, per-slot source-node ids
    (esrc), pad masks, permutations.
  - Launch A: per-core [Wh1^T; s_i; s_j] = ([W1 | W1@A1]^T x^T + bias) in a
    single fused fp16 matmul per 512-column chunk (f32 accumulate).
  - Between launches the host stages per-core inputs by pure indexing of
    device-computed values: slot-expanded fp16 features whe = Wh16[esrc] and
    pad-masked source scalars sje, so the device streams edge data with
    large sequential DMAs at full HBM bandwidth instead of per-edge gather
    descriptors.
  - Launch B (x2, one per GAT layer), phase 1 computes every attention
    weight up front: e = leaky(s_i + s_j + bA) on DVE+ACT over the whole
    slot grid, ex = exp(e) in fp16 (scores are O(5), no max-subtract
    needed), per-call denominator reduce, reciprocal x zero-degree flags.
    Phase 2 streams calls through a software pipeline: slot-chunk DMA ->
    message multiply split ~50/50 between the gpsimd and vector engines ->
    fp16 pairwise-tree slot reduction (2x DVE mode) -> per-tile scalar-engine
    Prelu fusing the softmax normalization (scale=1/den) with the leaky
    relu -> fp16 PE transpose -> fused epilogue matmul [Wn | Wn@An] giving
    the next layer's [Wh^T; s_i; s_j] (or the final fc output for layer 2).
"""

import dataclasses
import numpy as np

import concourse.bacc as bacc
import concourse.tile as tile
from concourse import bass, mybir, bass_utils
from concourse.masks import make_identity

F32 = mybir.dt.float32
F16 = mybir.dt.float16

N_NODES = 100000
N_CORES = 8
DPC = N_NODES // N_CORES
F = 64
IN_C = 128
NSUB = 1
CALL_W = 128  # max slot-columns per vector-op call
MERGE_SLACK = 1.25  # max Dmax/Dmin when merging tiles into one call
POOL_MULT_SHARE = 0.5  # fraction of message-multiply elems on gpsimd
POOL_TAPER = 0.1  # extra gpsimd share on the last POOL_TAPER_N calls
POOL_TAPER_N = 4
NEG_BIG = -1.0e30
EPS = 1e-16
ALPHA = 0.2


@dataclasses.dataclass
class Schedule:
    n_tiles: int
    w_total: int
    calls: list  # (sub, t0, ntc, D, col0)
    tilecol: np.ndarray  # int32 [w_total]: tile index of each slot column
    perms: list  # per core: int64 [n_tiles*128], local dst or -1
    esrc: list  # per core: int32 [128, w_total] global source id per slot (0 pad)
    emask: list  # per core: f32 [128, w_total] (0 real / NEG_BIG pad)
    flags: list  # per core: f32 [128, n_tiles]


def build_schedule(edge_index: np.ndarray) -> Schedule:
    src = np.asarray(edge_index[0], dtype=np.int64)
    dst = np.asarray(edge_index[1], dtype=np.int64)
    order = np.argsort(dst, kind="stable")
    src_s = src[order]
    deg_all = np.bincount(dst, minlength=N_NODES)
    starts_all = np.concatenate([[0], np.cumsum(deg_all)])

    # per-core sub-shard dst lists (round-robin over degree-sorted order)
    core_subs = []  # [core][sub] -> local dst ids
    for c in range(N_CORES):
        deg = deg_all[c * DPC : (c + 1) * DPC]
        rank = np.argsort(deg, kind="stable")
        core_subs.append([rank[s::NSUB] for s in range(NSUB)])

    # shared tile plan: per (sub, tile): D = max over cores of tile max-deg
    tiles = []
    for s in range(NSUB):
        nt = -(-max(len(core_subs[c][s]) for c in range(N_CORES)) // 128)
        for t in range(nt):
            mx = 1
            for c in range(N_CORES):
                lst = core_subs[c][s][t * 128 : (t + 1) * 128]
                if len(lst):
                    deg = deg_all[c * DPC + lst]
                    mx = max(mx, int(deg.max()))
            assert mx <= 512, mx  # a tile above CALL_W just becomes its own call
            tiles.append((s, mx))
    n_tiles = len(tiles)

    # call plan: merge consecutive same-sub tiles with similar D
    calls = []
    i = 0
    col = 0
    while i < n_tiles:
        s, D = tiles[i]
        Dmax = Dmin = D
        ntc = 1
        while i + ntc < n_tiles:
            s2, D2 = tiles[i + ntc]
            if s2 != s:
                break
            nD, mD = max(Dmax, D2), min(Dmin, D2)
            if (ntc + 1) * nD > CALL_W or nD > MERGE_SLACK * mD:
                break
            Dmax, Dmin = nD, mD
            ntc += 1
        calls.append((s, i, ntc, Dmax, col))
        col += ntc * Dmax
        i += ntc
    w_total = col
    tilecol = np.zeros(w_total, np.int32)
    for (s_, t0, ntc, D, col0) in calls:
        for tl in range(ntc):
            tilecol[col0 + tl * D : col0 + (tl + 1) * D] = t0 + tl

    perms, esrcs, emasks, flagss = [], [], [], []
    for c in range(N_CORES):
        perm = np.full(n_tiles * 128, -1, dtype=np.int64)
        ti = 0
        for s in range(NSUB):
            nt = sum(1 for (ss, _) in tiles if ss == s)
            lst = core_subs[c][s]
            block = np.full(nt * 128, -1, dtype=np.int64)
            block[: len(lst)] = lst
            perm[ti * 128 : (ti + nt) * 128] = block
            ti += nt

        esrc = np.zeros((128, w_total), np.int32)
        emask = np.full((128, w_total), np.float32(NEG_BIG))
        for (s, t0, ntc, D, col0) in calls:
            for tl in range(ntc):
                tglob = t0 + tl
                dsts = perm[tglob * 128 : (tglob + 1) * 128]
                for p in range(128):
                    d = dsts[p]
                    if d < 0:
                        continue
                    g = c * DPC + d
                    e0, ne = starts_all[g], deg_all[g]
                    c0 = col0 + tl * D
                    esrc[p, c0 : c0 + ne] = src_s[e0 : e0 + ne]
                    emask[p, c0 : c0 + ne] = 0.0
        pflat = perm.copy()
        okdeg = (pflat >= 0) & (deg_all[np.clip(c * DPC + pflat, 0, N_NODES - 1)] > 0)
        flags = np.ascontiguousarray(
            okdeg.reshape(n_tiles, 128).T.astype(np.float32)
        )
        perms.append(perm)
        esrcs.append(esrc)
        emasks.append(emask)
        flagss.append(flags)

    return Schedule(n_tiles, w_total, calls, tilecol, perms, esrcs, emasks, flagss)


# ---------------------------------------------------------------- prog A
def build_progA(n_loc=DPC, in_c=IN_C, f=F):
    nc = bacc.Bacc("TRN2", target_bir_lowering=False, debug=False, num_devices=N_CORES)
    xT = nc.dram_tensor("xT", [in_c, n_loc], F16, kind="ExternalInput").ap()
    Wf = nc.dram_tensor("Wf", [in_c, f + 2], F16, kind="ExternalInput").ap()
    bf = nc.dram_tensor("bf", [f + 2, 1], F32, kind="ExternalInput").ap()
    outT = nc.dram_tensor("outT", [f + 2, n_loc], F16, kind="ExternalOutput").ap()

    AF = mybir.ActivationFunctionType
    OP = mybir.AluOpType

    with tile.TileContext(nc) as tc:
        with tc.tile_pool(name="sb", bufs=1) as pool, tc.tile_pool(
            name="ps", bufs=4, space="PSUM"
        ) as pps, tc.tile_pool(name="sb2", bufs=4) as pool2:
            xT_sb = pool.tile([in_c, n_loc], F16)
            Wf_sb = pool.tile([in_c, f + 2], F16)
            nc.sync.dma_start(out=Wf_sb[:], in_=Wf[:, :])
            bf_sb = pool.tile([f + 2, 1], F32)
            nc.sync.dma_start(out=bf_sb[:], in_=bf[:, :])
            NSPL = 12
            spl = -(-n_loc // NSPL)
            for k in range(NSPL):
                a, b = k * spl, min(n_loc, (k + 1) * spl)
                nc.sync.dma_start(out=xT_sb[:, a:b], in_=xT[:, a:b])

            CH = 512
            GB = 6  # chunks per output DMA
            ob = None
            nch = -(-n_loc // CH)
            for ci, c0 in enumerate(range(0, n_loc, CH)):
                ch = min(CH, n_loc - c0)
                ps_w = pps.tile([f + 2, CH], F32, space="PSUM")
                nc.tensor.matmul(
                    out=ps_w[:, :ch],
                    lhsT=Wf_sb[:],
                    rhs=xT_sb[:, c0 : c0 + ch],
                    start=True,
                    stop=True,
                )
                g = ci % GB
                if g == 0:
                    ob = pool2.tile([f + 2, GB * CH], F16, tag="ob")
                if ci % 2 == 0:
                    nc.scalar.activation(
                        out=ob[:, g * CH : g * CH + ch],
                        in_=ps_w[:, :ch],
                        func=AF.Identity,
                        bias=bf_sb[:],
                    )
                else:
                    nc.vector.tensor_scalar(
                        out=ob[:, g * CH : g * CH + ch],
                        in0=ps_w[:, :ch],
                        scalar1=bf_sb[:],
                        scalar2=None,
                        op0=OP.add,
                    )
                if g == GB - 1 or ci == nch - 1:
                    b0 = (ci - g) * CH
                    nc.sync.dma_start(
                        out=outT[:, b0 : c0 + ch], in_=ob[:, : g * CH + ch]
                    )
    nc.compile()
    return nc


# ---------------------------------------------------------------- prog B
def build_progB(sched: Schedule, f=F):
    NT = sched.n_tiles
    WTOT = sched.w_total
    nc = bacc.Bacc(
        "TRN2",
        target_bir_lowering=False,
        debug=False,
        num_devices=N_CORES,
    )
    whe_d = nc.dram_tensor("whe", [128, WTOT * f], F16, kind="ExternalInput").ap()
    sje_d = nc.dram_tensor("sje", [128, WTOT], F32, kind="ExternalInput").ap()
    si_d = nc.dram_tensor("si", [128, NT], F32, kind="ExternalInput").ap()
    bA_d = nc.dram_tensor("bA", [128, 1], F32, kind="ExternalInput").ap()
    flags_d = nc.dram_tensor("flags", [128, NT], F32, kind="ExternalInput").ap()
    Wf_d = nc.dram_tensor("Wf", [f, f + 2], F16, kind="ExternalInput").ap()
    bf_d = nc.dram_tensor("bf", [f + 2, 1], F32, kind="ExternalInput").ap()
    outT_d = nc.dram_tensor("outT", [f + 2, NT * 128], F16, kind="ExternalOutput").ap()

    X = mybir.AxisListType.X
    AF = mybir.ActivationFunctionType
    OP = mybir.AluOpType

    def v(ap, dims, off=0):
        return dataclasses.replace(
            ap,
            ap=[list(ap.ap[0])] + [list(d) for d in dims],
            offset=ap.offset + off,
        )

    with tile.TileContext(nc) as tc:
        with tc.tile_pool(name="const", bufs=1) as pc, tc.tile_pool(
            name="io", bufs=4
        ) as pio, tc.tile_pool(name="work", bufs=3) as pw, tc.tile_pool(
            name="ps", bufs=2, space="PSUM"
        ) as pps, tc.tile_pool(name="ep", bufs=2) as pep:
            # critical-path consts first on the SP DMA queue, then the first
            # call's whe (Pool's multiply slice first so it can start while
            # the rest streams), then the consts only needed later
            sje_sb = pc.tile([128, WTOT], F32)
            nc.sync.dma_start(out=sje_sb[:], in_=sje_d[:, :])
            si_sb = pc.tile([128, NT], F32)
            nc.sync.dma_start(out=si_sb[:], in_=si_d[:, :])
            bA_sb = pc.tile([128, 1], F32)
            nc.sync.dma_start(out=bA_sb[:], in_=bA_d[:, :])
            (s0, t00, ntc0, D0, col00) = sched.calls[0]
            W0 = ntc0 * D0
            sh0 = POOL_MULT_SHARE + (
                POOL_TAPER if len(sched.calls) <= POOL_TAPER_N else 0.0
            )
            cp0 = max(1, min(W0 - 1, int(round(sh0 * W0))))
            whe0 = pio.tile([128, W0 * f], F16, tag="whe")
            nc.sync.dma_start(
                out=whe0[:, : cp0 * f], in_=whe_d[:, col00 * f : (col00 + cp0) * f]
            )
            nc.sync.dma_start(
                out=whe0[:, cp0 * f :],
                in_=whe_d[:, (col00 + cp0) * f : (col00 + W0) * f],
            )
            flags_sb = pc.tile([128, NT], F32)
            nc.sync.dma_start(out=flags_sb[:], in_=flags_d[:, :])
            Wf_sb = pc.tile([f, f + 2], F16)
            nc.sync.dma_start(out=Wf_sb[:], in_=Wf_d[:, :])
            bf_sb = pc.tile([f + 2, 1], F32)
            nc.sync.dma_start(out=bf_sb[:], in_=bf_d[:, :])
            ident = pc.tile([128, 128], F16)
            make_identity(nc, ident[:])

            # ---- phase 1: attention weights for the whole slot grid ----
            # e = leaky(si + sj + bA) (sj pre-masked to -inf on pad slots);
            # ex = exp(e)  (no max-subtract: scores are O(10) so exp fits
            # fp16/f32 comfortably)
            ep_sb = pc.tile([128, WTOT], F32)
            for (s, t0, ntc, D, col0) in sched.calls:
                nc.vector.tensor_tensor(
                    out=v(ep_sb[:], [(D, ntc), (1, D)], off=col0),
                    in0=v(sje_sb[:], [(D, ntc), (1, D)], off=col0),
                    in1=si_sb[:, t0 : t0 + ntc].to_broadcast([128, ntc, D]),
                    op=OP.add,
                )
            ex16 = pc.tile([128, WTOT], F16)
            nc.scalar.activation(
                out=ep_sb[:], in_=ep_sb[:], func=AF.Prelu, alpha=ALPHA, bias=bA_sb[:]
            )
            nc.scalar.activation(out=ex16[:], in_=ep_sb[:], func=AF.Exp)
            den = pc.tile([128, NT], F32)
            rden = pc.tile([128, NT], F32)

            def denominators():
                for (s, t0, ntc, D, col0) in sched.calls:
                    nc.vector.tensor_reduce(
                        out=den[:, t0 : t0 + ntc],
                        in_=v(ex16[:], [(D, ntc), (1, D)], off=col0),
                        axis=X,
                        op=OP.add,
                    )
                nc.vector.tensor_scalar(
                    out=den[:], in0=den[:], scalar1=EPS, scalar2=None, op0=OP.add
                )
                nc.vector.reciprocal(out=rden[:], in_=den[:])
                nc.vector.tensor_tensor(
                    out=rden[:], in0=rden[:], in1=flags_sb[:], op=OP.mult
                )

            # ---- phase 2: stream messages, reduce, epilogue ----
            CHT = 4  # tiles per epilogue chunk (512 dsts)

            def flush_chunk(ts, ntl, hTL, ob):
                cols = ntl * 128
                ps_w = pps.tile([f + 2, CHT * 128], F32, tag="psw", space="PSUM")
                nc.tensor.matmul(
                    out=ps_w[:, :cols],
                    lhsT=Wf_sb[:],
                    rhs=hTL[:, :cols],
                    start=True,
                    stop=True,
                )
                nc.scalar.activation(
                    out=ob[:, :cols],
                    in_=ps_w[:, :cols],
                    func=AF.Identity,
                    bias=bf_sb[:],
                )
                nc.sync.dma_start(
                    out=outT_d[:, ts * 128 : ts * 128 + cols],
                    in_=ob[:, :cols],
                )

            def stage1(ci, whe=None):
                (s, t0, ntc, D, col0) = sched.calls[ci]
                W = ntc * D
                sh = POOL_MULT_SHARE + (
                    POOL_TAPER if ci >= len(sched.calls) - POOL_TAPER_N else 0.0
                )
                cp = max(1, min(W - 1, int(round(sh * W))))
                if whe is None:
                    whe = pio.tile([128, W * f], F16, tag="whe")
                    nc.sync.dma_start(
                        out=whe[:], in_=whe_d[:, col0 * f : (col0 + W) * f]
                    )
                # weighted message (fp16): msg = Wh16 * ex16; each call's
                # multiply is split between gpsimd and DVE at the balance point
                msg16 = pw.tile([128, W * f], F16, tag="msg16")
                nc.gpsimd.tensor_tensor(
                    out=v(msg16[:], [(f, cp), (1, f)]),
                    in0=v(whe[:], [(f, cp), (1, f)]),
                    in1=ex16[:, col0 : col0 + cp].to_broadcast([128, cp, f]),
                    op=OP.mult,
                )
                nc.vector.tensor_tensor(
                    out=v(msg16[:], [(f, W - cp), (1, f)], off=cp * f),
                    in0=v(whe[:], [(f, W - cp), (1, f)], off=cp * f),
                    in1=ex16[:, col0 + cp : col0 + W].to_broadcast(
                        [128, W - cp, f]
                    ),
                    op=OP.mult,
                )
                return msg16

            def stage2(ci, msg16):
                (s, t0, ntc, D, col0) = sched.calls[ci]
                W = ntc * D
                # pairwise-tree reduce over the D slots (fp16 packed -> 2x DVE)
                hraw = pw.tile([128, ntc * f], F32, tag="hraw")
                if D == 1:
                    nc.vector.tensor_copy(
                        out=v(hraw[:], [(f, ntc), (1, f)]),
                        in_=v(msg16[:], [(D * f, ntc), (1, f)]),
                    )
                else:
                    p2 = 1
                    while p2 * 2 <= D:
                        p2 *= 2
                    if D > p2:
                        r = D - p2
                        nc.vector.tensor_tensor(
                            out=v(msg16[:], [(D * f, ntc), (1, r * f)]),
                            in0=v(msg16[:], [(D * f, ntc), (1, r * f)]),
                            in1=v(msg16[:], [(D * f, ntc), (1, r * f)], off=p2 * f),
                            op=OP.add,
                        )
                    while p2 > 2:
                        h = p2 // 2
                        nc.vector.tensor_tensor(
                            out=v(msg16[:], [(D * f, ntc), (1, h * f)]),
                            in0=v(msg16[:], [(D * f, ntc), (1, h * f)]),
                            in1=v(msg16[:], [(D * f, ntc), (1, h * f)], off=h * f),
                            op=OP.add,
                        )
                        p2 = h
                    nc.vector.tensor_tensor(
                        out=v(hraw[:], [(f, ntc), (1, f)]),
                        in0=v(msg16[:], [(D * f, ntc), (1, f)]),
                        in1=v(msg16[:], [(D * f, ntc), (1, f)], off=f),
                        op=OP.add,
                    )
                # h = leaky(hraw * rden): per-tile ACT fuses the softmax
                # normalization (scale) with the leaky relu; the last two
                # calls run it on the (by then idle) DVE to shorten the tail
                tail_call = ci >= len(sched.calls) - 2
                if tail_call:
                    nc.vector.tensor_tensor(
                        out=v(hraw[:], [(f, ntc), (1, f)]),
                        in0=v(hraw[:], [(f, ntc), (1, f)]),
                        in1=rden[:, t0 : t0 + ntc].to_broadcast([128, ntc, f]),
                        op=OP.mult,
                    )
                    hl_all = pw.tile([128, ntc * f], F16, tag="hl_all")
                    nc.vector.scalar_tensor_tensor(
                        out=hl_all[:],
                        in0=hraw[:],
                        scalar=ALPHA,
                        in1=hraw[:],
                        op0=OP.mult,
                        op1=OP.max,
                    )
                ps_chunk = None
                for tl in range(ntc):
                    t = t0 + tl
                    if tail_call:
                        hl = hl_all[:, tl * f : (tl + 1) * f]
                    else:
                        hlt = pw.tile([128, f], F16, tag="hl")
                        nc.scalar.activation(
                            out=hlt[:],
                            in_=hraw[:, tl * f : (tl + 1) * f],
                            func=AF.Prelu,
                            alpha=ALPHA,
                            scale=rden[:, t : t + 1],
                        )
                        hl = hlt[:]
                    j = tl % CHT
                    if j == 0:
                        ps_chunk = pps.tile(
                            [f, CHT * 128], F16, tag="pst", space="PSUM"
                        )
                        hTL = pep.tile([f, CHT * 128], F16, tag="hTL")
                        ob = pep.tile([f + 2, CHT * 128], F16, tag="ob")
                    # transposes accumulate into one PSUM chunk; a single
                    # copy then moves the whole chunk to SBUF for the matmul
                    nc.tensor.transpose(
                        out=ps_chunk[:, j * 128 : (j + 1) * 128],
                        in_=hl,
                        identity=ident[:],
                    )
                    if j == CHT - 1 or tl == ntc - 1:
                        cols = (j + 1) * 128
                        nc.scalar.activation(
                            out=hTL[:, :cols],
                            in_=ps_chunk[:, :cols],
                            func=AF.Identity,
                        )
                        flush_chunk(t - j, j + 1, hTL, ob)

            # 2-stage software pipeline: issue call ci+1's DMA+multiplies
            # before call ci's reduce/epilogue so the in-order DVE queue
            # never blocks the next multiply behind a Pool-gated reduce
            ncalls = len(sched.calls)
            prev = stage1(0, whe0)
            denominators()  # after call 0's multiply so DVE starts it sooner
            for ci in range(1, ncalls):
                cur = stage1(ci)
                stage2(ci - 1, prev)
                prev = cur
            stage2(ncalls - 1, prev)
    nc.compile()
    return nc


# ---------------------------------------------------------------- driver
_cache = {}
last_results = []  # BassKernelResults per launch (for test.py profiling)


def kernel(x, edge_index, W1, bW1, A1, bA1, W2, bW2, A2, bA2, Wfc, bfc):
    x = np.asarray(x, dtype=np.float32)
    edge_index = np.asarray(edge_index)
    W1 = np.asarray(W1, np.float32)
    bW1 = np.asarray(bW1, np.float32)
    A1 = np.asarray(A1, np.float32)
    bA1 = np.asarray(bA1, np.float32)
    W2 = np.asarray(W2, np.float32)
    bW2 = np.asarray(bW2, np.float32)
    A2 = np.asarray(A2, np.float32)
    bA2 = np.asarray(bA2, np.float32)
    Wfc = np.asarray(Wfc, np.float32)
    bfc = np.asarray(bfc, np.float32)

    sched = build_schedule(edge_index)
    cores = list(range(N_CORES))
    last_results.clear()

    if "A" not in _cache:
        _cache["A"] = build_progA()
    ncA = _cache["A"]
    As1 = np.ascontiguousarray(np.concatenate([A1[:F], A1[F:]], axis=1))
    Wf1 = np.concatenate([W1, W1 @ As1], axis=1).astype(np.float16)
    bf1 = np.concatenate(
        [bW1.reshape(F, 1), As1.T @ bW1.reshape(F, 1)], axis=0
    ).astype(np.float32)
    inA = []
    for c in cores:
        xT = np.ascontiguousarray(x[c * DPC : (c + 1) * DPC].T.astype(np.float16))
        inA.append({"xT": xT, "Wf": Wf1, "bf": bf1})
    resA = bass_utils.run_bass_kernel_spmd(ncA, inA, core_ids=cores)
    last_results.append(resA)
    wh = np.concatenate(
        [resA.results[c]["outT"][:F].T.astype(np.float32) for c in cores], axis=0
    )
    s_all = np.concatenate(
        [resA.results[c]["outT"][F : F + 2].astype(np.float32) for c in cores], axis=1
    )
    si_full, sj_full = s_all[0], s_all[1]

    key = ("B", sched.n_tiles, sched.w_total, tuple(sched.calls))
    if key not in _cache:
        _cache[key] = build_progB(sched)
    ncB = _cache[key]

    def launch_B(wh_full, si_f, sj_f, bA, Wn, bWn, An):
        wh16 = wh_full.astype(np.float16)
        Wfn = np.concatenate([Wn, Wn @ An], axis=1).astype(np.float16)
        bfn = np.concatenate(
            [bWn.reshape(F, 1), An.T @ bWn.reshape(F, 1)], axis=0
        ).astype(np.float32)
        inB = []
        for c in cores:
            perm = sched.perms[c]
            real = perm >= 0
            gids = c * DPC + perm[real]
            tmp = np.zeros(sched.n_tiles * 128, np.float32)
            tmp[real] = si_f[gids]
            si_arr = tmp.reshape(sched.n_tiles, 128).T
            esrc = sched.esrc[c]
            whe = wh16[esrc.ravel()].reshape(128, sched.w_total * F)
            sje = np.where(
                sched.emask[c] < 0.0, np.float32(NEG_BIG), sj_f[esrc]
            ).astype(np.float32)
            inB.append(
                {
                    "whe": whe,
                    "sje": sje,
                    "si": np.ascontiguousarray(si_arr),
                    "bA": np.full((128, 1), bA.reshape(-1)[0], np.float32),
                    "flags": sched.flags[c],
                    "Wf": Wfn,
                    "bf": bfn,
                }
            )
        res = bass_utils.run_bass_kernel_spmd(ncB, inB, core_ids=cores)
        last_results.append(res)
        whn = np.zeros((N_NODES, F), np.float32)
        sn_i = np.zeros(N_NODES, np.float32)
        sn_j = np.zeros(N_NODES, np.float32)
        for c in cores:
            perm = sched.perms[c]
            real = perm >= 0
            gids = c * DPC + perm[real]
            outT = res.results[c]["outT"].astype(np.float32)
            whn[gids] = outT[:F].T[real]
            sn_i[gids] = outT[F][real]
            sn_j[gids] = outT[F + 1][real]
        return whn, sn_i, sn_j

    As2 = np.ascontiguousarray(np.concatenate([A2[:F], A2[F:]], axis=1))
    wh2, si2, sj2 = launch_B(wh, si_full, sj_full, bA1, W2, bW2, As2)
    out, _, _ = launch_B(wh2, si2, sj2, bA2, Wfc, bfc, np.zeros((F, 2), np.float32))
    return out.astype(np.float32)


# revision 51
# speedup vs baseline: 1.0446x; 1.0021x over previous
"""GAT (2-layer) on 8 NeuronCores — Bass/Tile kernel.

Strategy (dst-sharded graph parallel):
  - Each core owns 12500 destination nodes, degree-sorted and tiled into
    128-dst tiles on a dense slot grid (slot width = exact per-tile max
    degree over all cores); adjacent similar-degree tiles are merged into
    <=128-column calls (Dmax/Dmin <= 1.25 slack).
  - Host pre-pass is index-only: the slot grid, per-slot source-node ids
    (esrc), pad masks, permutations.
  - Launch A: per-core [Wh1^T; s_i; s_j] = ([W1 | W1@A1]^T x^T + bias) in a
    single fused fp16 matmul per 512-column chunk (f32 accumulate).
  - Between launches the host stages per-core inputs by pure indexing of
    device-computed values: slot-expanded fp16 features whe = Wh16[esrc] and
    pad-masked source scalars sje, so the device streams edge data with
    large sequential DMAs at full HBM bandwidth instead of per-edge gather
    descriptors.
  - Launch B (x2, one per GAT layer), phase 1 computes every attention
    weight up front: e = leaky(s_i + s_j + bA) on DVE+ACT over the whole
    slot grid, ex = exp(e) in fp16 (scores are O(5), no max-subtract
    needed), per-call denominator reduce, reciprocal x zero-degree flags.
    Phase 2 streams calls through a software pipeline: slot-chunk DMA ->
    message multiply split ~50/50 between the gpsimd and vector engines ->
    fp16 pairwise-tree slot reduction (2x DVE mode) -> per-tile scalar-engine
    Prelu fusing the softmax normalization (scale=1/den) with the leaky
    relu -> fp16 PE transpose -> fused epilogue matmul [Wn | Wn@An] giving
    the next layer's [Wh^T; s_i; s_j] (or the final fc output for layer 2).
"""

import dataclasses
import numpy as np

import concourse.bacc as bacc
import concourse.tile as tile
from concourse import bass, mybir, bass_utils
from concourse.masks import make_identity

F32 = mybir.dt.float32
F16 = mybir.dt.float16

N_NODES = 100000
N_CORES = 8
DPC = N_NODES // N_CORES
F = 64
IN_C = 128
NSUB = 1
CALL_W = 128  # max slot-columns per vector-op call
MERGE_SLACK = 1.25  # max Dmax/Dmin when merging tiles into one call
POOL_MULT_SHARE = 0.5  # fraction of message-multiply elems on gpsimd
POOL_TAPER = 0.1  # extra gpsimd share on the last POOL_TAPER_N calls
POOL_TAPER_N = 4
NEG_BIG = -1.0e30
EPS = 1e-16
ALPHA = 0.2


@dataclasses.dataclass
class Schedule:
    n_tiles: int
    w_total: int
    calls: list  # (sub, t0, ntc, D, col0)
    tilecol: np.ndarray  # int32 [w_total]: tile index of each slot column
    perms: list  # per core: int64 [n_tiles*128], local dst or -1
    esrc: list  # per core: int32 [128, w_total] global source id per slot (0 pad)
    emask: list  # per core: f32 [128, w_total] (0 real / NEG_BIG pad)
    flags: list  # per core: f32 [128, n_tiles]


def build_schedule(edge_index: np.ndarray) -> Schedule:
    src = np.asarray(edge_index[0], dtype=np.int64)
    dst = np.asarray(edge_index[1], dtype=np.int64)
    order = np.argsort(dst, kind="stable")
    src_s = src[order]
    deg_all = np.bincount(dst, minlength=N_NODES)
    starts_all = np.concatenate([[0], np.cumsum(deg_all)])

    # per-core sub-shard dst lists (round-robin over degree-sorted order)
    core_subs = []  # [core][sub] -> local dst ids
    for c in range(N_CORES):
        deg = deg_all[c * DPC : (c + 1) * DPC]
        rank = np.argsort(deg, kind="stable")
        core_subs.append([rank[s::NSUB] for s in range(NSUB)])

    # shared tile plan: per (sub, tile): D = max over cores of tile max-deg
    tiles = []
    for s in range(NSUB):
        nt = -(-max(len(core_subs[c][s]) for c in range(N_CORES)) // 128)
        for t in range(nt):
            mx = 1
            for c in range(N_CORES):
                lst = core_subs[c][s][t * 128 : (t + 1) * 128]
                if len(lst):
                    deg = deg_all[c * DPC + lst]
                    mx = max(mx, int(deg.max()))
            assert mx <= 512, mx  # a tile above CALL_W just becomes its own call
            tiles.append((s, mx))
    n_tiles = len(tiles)

    # call plan: merge consecutive same-sub tiles with similar D
    calls = []
    i = 0
    col = 0
    while i < n_tiles:
        s, D = tiles[i]
        Dmax = Dmin = D
        ntc = 1
        while i + ntc < n_tiles:
            s2, D2 = tiles[i + ntc]
            if s2 != s:
                break
            nD, mD = max(Dmax, D2), min(Dmin, D2)
            if (ntc + 1) * nD > CALL_W or nD > MERGE_SLACK * mD:
                break
            Dmax, Dmin = nD, mD
            ntc += 1
        calls.append((s, i, ntc, Dmax, col))
        col += ntc * Dmax
        i += ntc
    w_total = col
    tilecol = np.zeros(w_total, np.int32)
    for (s_, t0, ntc, D, col0) in calls:
        for tl in range(ntc):
            tilecol[col0 + tl * D : col0 + (tl + 1) * D] = t0 + tl

    perms, esrcs, emasks, flagss = [], [], [], []
    for c in range(N_CORES):
        perm = np.full(n_tiles * 128, -1, dtype=np.int64)
        ti = 0
        for s in range(NSUB):
            nt = sum(1 for (ss, _) in tiles if ss == s)
            lst = core_subs[c][s]
            block = np.full(nt * 128, -1, dtype=np.int64)
            block[: len(lst)] = lst
            perm[ti * 128 : (ti + nt) * 128] = block
            ti += nt

        esrc = np.zeros((128, w_total), np.int32)
        emask = np.full((128, w_total), np.float32(NEG_BIG))
        for (s, t0, ntc, D, col0) in calls:
            for tl in range(ntc):
                tglob = t0 + tl
                dsts = perm[tglob * 128 : (tglob + 1) * 128]
                for p in range(128):
                    d = dsts[p]
                    if d < 0:
                        continue
                    g = c * DPC + d
                    e0, ne = starts_all[g], deg_all[g]
                    c0 = col0 + tl * D
                    esrc[p, c0 : c0 + ne] = src_s[e0 : e0 + ne]
                    emask[p, c0 : c0 + ne] = 0.0
        pflat = perm.copy()
        okdeg = (pflat >= 0) & (deg_all[np.clip(c * DPC + pflat, 0, N_NODES - 1)] > 0)
        flags = np.ascontiguousarray(
            okdeg.reshape(n_tiles, 128).T.astype(np.float32)
        )
        perms.append(perm)
        esrcs.append(esrc)
        emasks.append(emask)
        flagss.append(flags)

    return Schedule(n_tiles, w_total, calls, tilecol, perms, esrcs, emasks, flagss)


# ---------------------------------------------------------------- prog A
def build_progA(n_loc=DPC, in_c=IN_C, f=F):
    nc = bacc.Bacc("TRN2", target_bir_lowering=False, debug=False, num_devices=N_CORES)
    xT = nc.dram_tensor("xT", [in_c, n_loc], F16, kind="ExternalInput").ap()
    Wf = nc.dram_tensor("Wf", [in_c, f + 2], F16, kind="ExternalInput").ap()
    bf = nc.dram_tensor("bf", [f + 2, 1], F32, kind="ExternalInput").ap()
    outT = nc.dram_tensor("outT", [f + 2, n_loc], F16, kind="ExternalOutput").ap()

    AF = mybir.ActivationFunctionType
    OP = mybir.AluOpType

    with tile.TileContext(nc) as tc:
        with tc.tile_pool(name="sb", bufs=1) as pool, tc.tile_pool(
            name="ps", bufs=4, space="PSUM"
        ) as pps, tc.tile_pool(name="sb2", bufs=4) as pool2:
            xT_sb = pool.tile([in_c, n_loc], F16)
            Wf_sb = pool.tile([in_c, f + 2], F16)
            nc.sync.dma_start(out=Wf_sb[:], in_=Wf[:, :])
            bf_sb = pool.tile([f + 2, 1], F32)
            nc.sync.dma_start(out=bf_sb[:], in_=bf[:, :])
            NSPL = 12
            spl = -(-n_loc // NSPL)
            for k in range(NSPL):
                a, b = k * spl, min(n_loc, (k + 1) * spl)
                nc.sync.dma_start(out=xT_sb[:, a:b], in_=xT[:, a:b])

            CH = 512
            GB = 6  # chunks per output DMA
            ob = None
            nch = -(-n_loc // CH)
            for ci, c0 in enumerate(range(0, n_loc, CH)):
                ch = min(CH, n_loc - c0)
                ps_w = pps.tile([f + 2, CH], F32, space="PSUM")
                nc.tensor.matmul(
                    out=ps_w[:, :ch],
                    lhsT=Wf_sb[:],
                    rhs=xT_sb[:, c0 : c0 + ch],
                    start=True,
                    stop=True,
                )
                g = ci % GB
                if g == 0:
                    ob = pool2.tile([f + 2, GB * CH], F16, tag="ob")
                if ci % 2 == 0:
                    nc.scalar.activation(
                        out=ob[:, g * CH : g * CH + ch],
                        in_=ps_w[:, :ch],
                        func=AF.Identity,
                        bias=bf_sb[:],
                    )
                else:
                    nc.vector.tensor_scalar(
                        out=ob[:, g * CH : g * CH + ch],
                        in0=ps_w[:, :ch],
                        scalar1=bf_sb[:],
                        scalar2=None,
                        op0=OP.add,
                    )
                if g == GB - 1 or ci == nch - 1:
                    b0 = (ci - g) * CH
                    nc.sync.dma_start(
                        out=outT[:, b0 : c0 + ch], in_=ob[:, : g * CH + ch]
                    )
    nc.compile()
    return nc


# ---------------------------------------------------------------- prog B
def build_progB(sched: Schedule, f=F):
    NT = sched.n_tiles
    WTOT = sched.w_total
    nc = bacc.Bacc(
        "TRN2",
        target_bir_lowering=False,
        debug=False,
        num_devices=N_CORES,
    )
    whe_d = nc.dram_tensor("whe", [128, WTOT * f], F16, kind="ExternalInput").ap()
    sje_d = nc.dram_tensor("sje", [128, WTOT], F32, kind="ExternalInput").ap()
    si_d = nc.dram_tensor("si", [128, NT], F32, kind="ExternalInput").ap()
    bA_d = nc.dram_tensor("bA", [128, 1], F32, kind="ExternalInput").ap()
    flags_d = nc.dram_tensor("flags", [128, NT], F32, kind="ExternalInput").ap()
    Wf_d = nc.dram_tensor("Wf", [f, f + 2], F16, kind="ExternalInput").ap()
    bf_d = nc.dram_tensor("bf", [f + 2, 1], F32, kind="ExternalInput").ap()
    outT_d = nc.dram_tensor("outT", [f + 2, NT * 128], F16, kind="ExternalOutput").ap()

    X = mybir.AxisListType.X
    AF = mybir.ActivationFunctionType
    OP = mybir.AluOpType

    def v(ap, dims, off=0):
        return dataclasses.replace(
            ap,
            ap=[list(ap.ap[0])] + [list(d) for d in dims],
            offset=ap.offset + off,
        )

    with tile.TileContext(nc) as tc:
        with tc.tile_pool(name="const", bufs=1) as pc, tc.tile_pool(
            name="io", bufs=4
        ) as pio, tc.tile_pool(name="work", bufs=3) as pw, tc.tile_pool(
            name="ps", bufs=2, space="PSUM"
        ) as pps, tc.tile_pool(name="ep", bufs=2) as pep:
            # critical-path consts first on the SP DMA queue, then the first
            # call's whe (Pool's multiply slice first so it can start while
            # the rest streams), then the consts only needed later
            sje_sb = pc.tile([128, WTOT], F32)
            nc.sync.dma_start(out=sje_sb[:], in_=sje_d[:, :])
            si_sb = pc.tile([128, NT], F32)
            nc.sync.dma_start(out=si_sb[:], in_=si_d[:, :])
            bA_sb = pc.tile([128, 1], F32)
            nc.sync.dma_start(out=bA_sb[:], in_=bA_d[:, :])
            (s0, t00, ntc0, D0, col00) = sched.calls[0]
            W0 = ntc0 * D0
            sh0 = POOL_MULT_SHARE + (
                POOL_TAPER if len(sched.calls) <= POOL_TAPER_N else 0.0
            )
            cp0 = max(1, min(W0 - 1, int(round(sh0 * W0))))
            whe0 = pio.tile([128, W0 * f], F16, tag="whe")
            nc.sync.dma_start(
                out=whe0[:, : cp0 * f], in_=whe_d[:, col00 * f : (col00 + cp0) * f]
            )
            nc.sync.dma_start(
                out=whe0[:, cp0 * f :],
                in_=whe_d[:, (col00 + cp0) * f : (col00 + W0) * f],
            )
            flags_sb = pc.tile([128, NT], F32)
            nc.sync.dma_start(out=flags_sb[:], in_=flags_d[:, :])
            Wf_sb = pc.tile([f, f + 2], F16)
            nc.sync.dma_start(out=Wf_sb[:], in_=Wf_d[:, :])
            bf_sb = pc.tile([f + 2, 1], F32)
            nc.sync.dma_start(out=bf_sb[:], in_=bf_d[:, :])
            ident = pc.tile([128, 128], F16)
            make_identity(nc, ident[:])

            # ---- phase 1: attention weights for the whole slot grid ----
            # e = leaky(si + sj + bA) (sj pre-masked to -inf on pad slots);
            # ex = exp(e)  (no max-subtract: scores are O(10) so exp fits
            # fp16/f32 comfortably)
            ep_sb = pc.tile([128, WTOT], F32)
            for (s, t0, ntc, D, col0) in sched.calls:
                nc.vector.tensor_tensor(
                    out=v(ep_sb[:], [(D, ntc), (1, D)], off=col0),
                    in0=v(sje_sb[:], [(D, ntc), (1, D)], off=col0),
                    in1=si_sb[:, t0 : t0 + ntc].to_broadcast([128, ntc, D]),
                    op=OP.add,
                )
            ex16 = pc.tile([128, WTOT], F16)
            nc.scalar.activation(
                out=ep_sb[:], in_=ep_sb[:], func=AF.Prelu, alpha=ALPHA, bias=bA_sb[:]
            )
            nc.scalar.activation(out=ex16[:], in_=ep_sb[:], func=AF.Exp)
            den = pc.tile([128, NT], F32)
            rden = pc.tile([128, NT], F32)

            def denominators():
                for (s, t0, ntc, D, col0) in sched.calls:
                    nc.vector.tensor_reduce(
                        out=den[:, t0 : t0 + ntc],
                        in_=v(ex16[:], [(D, ntc), (1, D)], off=col0),
                        axis=X,
                        op=OP.add,
                    )
                nc.vector.tensor_scalar(
                    out=den[:], in0=den[:], scalar1=EPS, scalar2=None, op0=OP.add
                )
                nc.vector.reciprocal(out=rden[:], in_=den[:])
                nc.vector.tensor_tensor(
                    out=rden[:], in0=rden[:], in1=flags_sb[:], op=OP.mult
                )

            # ---- phase 2: stream messages, reduce, epilogue ----
            CHT = 8  # tiles per epilogue chunk (1024 dsts)

            def flush_chunk(ts, ntl, hTL, ob):
                cols = ntl * 128
                ps_w = pps.tile([f + 2, CHT * 128], F32, tag="psw", space="PSUM")
                for q0 in range(0, cols, 512):
                    qw = min(512, cols - q0)
                    nc.tensor.matmul(
                        out=ps_w[:, q0 : q0 + qw],
                        lhsT=Wf_sb[:],
                        rhs=hTL[:, q0 : q0 + qw],
                        start=True,
                        stop=True,
                    )
                nc.scalar.activation(
                    out=ob[:, :cols],
                    in_=ps_w[:, :cols],
                    func=AF.Identity,
                    bias=bf_sb[:],
                )
                nc.sync.dma_start(
                    out=outT_d[:, ts * 128 : ts * 128 + cols],
                    in_=ob[:, :cols],
                )

            def stage1(ci, whe=None):
                (s, t0, ntc, D, col0) = sched.calls[ci]
                W = ntc * D
                sh = POOL_MULT_SHARE + (
                    POOL_TAPER if ci >= len(sched.calls) - POOL_TAPER_N else 0.0
                )
                cp = max(1, min(W - 1, int(round(sh * W))))
                if whe is None:
                    whe = pio.tile([128, W * f], F16, tag="whe")
                    nc.sync.dma_start(
                        out=whe[:], in_=whe_d[:, col0 * f : (col0 + W) * f]
                    )
                # weighted message (fp16): msg = Wh16 * ex16; each call's
                # multiply is split between gpsimd and DVE at the balance point
                msg16 = pw.tile([128, W * f], F16, tag="msg16")
                nc.gpsimd.tensor_tensor(
                    out=v(msg16[:], [(f, cp), (1, f)]),
                    in0=v(whe[:], [(f, cp), (1, f)]),
                    in1=ex16[:, col0 : col0 + cp].to_broadcast([128, cp, f]),
                    op=OP.mult,
                )
                nc.vector.tensor_tensor(
                    out=v(msg16[:], [(f, W - cp), (1, f)], off=cp * f),
                    in0=v(whe[:], [(f, W - cp), (1, f)], off=cp * f),
                    in1=ex16[:, col0 + cp : col0 + W].to_broadcast(
                        [128, W - cp, f]
                    ),
                    op=OP.mult,
                )
                return msg16

            def stage2(ci, msg16):
                (s, t0, ntc, D, col0) = sched.calls[ci]
                W = ntc * D
                # pairwise-tree reduce over the D slots (fp16 packed -> 2x DVE)
                hraw = pw.tile([128, ntc * f], F32, tag="hraw")
                if D == 1:
                    nc.vector.tensor_copy(
                        out=v(hraw[:], [(f, ntc), (1, f)]),
                        in_=v(msg16[:], [(D * f, ntc), (1, f)]),
                    )
                else:
                    p2 = 1
                    while p2 * 2 <= D:
                        p2 *= 2
                    if D > p2:
                        r = D - p2
                        nc.vector.tensor_tensor(
                            out=v(msg16[:], [(D * f, ntc), (1, r * f)]),
                            in0=v(msg16[:], [(D * f, ntc), (1, r * f)]),
                            in1=v(msg16[:], [(D * f, ntc), (1, r * f)], off=p2 * f),
                            op=OP.add,
                        )
                    while p2 > 2:
                        h = p2 // 2
                        nc.vector.tensor_tensor(
                            out=v(msg16[:], [(D * f, ntc), (1, h * f)]),
                            in0=v(msg16[:], [(D * f, ntc), (1, h * f)]),
                            in1=v(msg16[:], [(D * f, ntc), (1, h * f)], off=h * f),
                            op=OP.add,
                        )
                        p2 = h
                    nc.vector.tensor_tensor(
                        out=v(hraw[:], [(f, ntc), (1, f)]),
                        in0=v(msg16[:], [(D * f, ntc), (1, f)]),
                        in1=v(msg16[:], [(D * f, ntc), (1, f)], off=f),
                        op=OP.add,
                    )
                # h = leaky(hraw * rden): per-tile ACT fuses the softmax
                # normalization (scale) with the leaky relu; the last two
                # calls run it on the (by then idle) DVE to shorten the tail
                tail_call = ci >= len(sched.calls) - 2
                if tail_call:
                    nc.vector.tensor_tensor(
                        out=v(hraw[:], [(f, ntc), (1, f)]),
                        in0=v(hraw[:], [(f, ntc), (1, f)]),
                        in1=rden[:, t0 : t0 + ntc].to_broadcast([128, ntc, f]),
                        op=OP.mult,
                    )
                    hl_all = pw.tile([128, ntc * f], F16, tag="hl_all")
                    nc.vector.scalar_tensor_tensor(
                        out=hl_all[:],
                        in0=hraw[:],
                        scalar=ALPHA,
                        in1=hraw[:],
                        op0=OP.mult,
                        op1=OP.max,
                    )
                ps_chunk = None
                for tl in range(ntc):
                    t = t0 + tl
                    if tail_call:
                        hl = hl_all[:, tl * f : (tl + 1) * f]
                    else:
                        hlt = pw.tile([128, f], F16, tag="hl")
                        nc.scalar.activation(
                            out=hlt[:],
                            in_=hraw[:, tl * f : (tl + 1) * f],
                            func=AF.Prelu,
                            alpha=ALPHA,
                            scale=rden[:, t : t + 1],
                        )
                        hl = hlt[:]
                    j = tl % CHT
                    if j == 0:
                        ps_chunk = pps.tile(
                            [f, CHT * 128], F16, tag="pst", space="PSUM"
                        )
                        hTL = pep.tile([f, CHT * 128], F16, tag="hTL")
                        ob = pep.tile([f + 2, CHT * 128], F16, tag="ob")
                    # transposes accumulate into one PSUM chunk; a single
                    # copy then moves the whole chunk to SBUF for the matmul
                    nc.tensor.transpose(
                        out=ps_chunk[:, j * 128 : (j + 1) * 128],
                        in_=hl,
                        identity=ident[:],
                    )
                    if j == CHT - 1 or tl == ntc - 1:
                        cols = (j + 1) * 128
                        nc.scalar.activation(
                            out=hTL[:, :cols],
                            in_=ps_chunk[:, :cols],
                            func=AF.Identity,
                        )
                        flush_chunk(t - j, j + 1, hTL, ob)

            # 2-stage software pipeline: issue call ci+1's DMA+multiplies
            # before call ci's reduce/epilogue so the in-order DVE queue
            # never blocks the next multiply behind a Pool-gated reduce
            ncalls = len(sched.calls)
            prev = stage1(0, whe0)
            denominators()  # after call 0's multiply so DVE starts it sooner
            for ci in range(1, ncalls):
                cur = stage1(ci)
                stage2(ci - 1, prev)
                prev = cur
            stage2(ncalls - 1, prev)
    nc.compile()
    return nc


# ---------------------------------------------------------------- driver
_cache = {}
last_results = []  # BassKernelResults per launch (for test.py profiling)


def kernel(x, edge_index, W1, bW1, A1, bA1, W2, bW2, A2, bA2, Wfc, bfc):
    x = np.asarray(x, dtype=np.float32)
    edge_index = np.asarray(edge_index)
    W1 = np.asarray(W1, np.float32)
    bW1 = np.asarray(bW1, np.float32)
    A1 = np.asarray(A1, np.float32)
    bA1 = np.asarray(bA1, np.float32)
    W2 = np.asarray(W2, np.float32)
    bW2 = np.asarray(bW2, np.float32)
    A2 = np.asarray(A2, np.float32)
    bA2 = np.asarray(bA2, np.float32)
    Wfc = np.asarray(Wfc, np.float32)
    bfc = np.asarray(bfc, np.float32)

    sched = build_schedule(edge_index)
    cores = list(range(N_CORES))
    last_results.clear()

    if "A" not in _cache:
        _cache["A"] = build_progA()
    ncA = _cache["A"]
    As1 = np.ascontiguousarray(np.concatenate([A1[:F], A1[F:]], axis=1))
    Wf1 = np.concatenate([W1, W1 @ As1], axis=1).astype(np.float16)
    bf1 = np.concatenate(
        [bW1.reshape(F, 1), As1.T @ bW1.reshape(F, 1)], axis=0
    ).astype(np.float32)
    inA = []
    for c in cores:
        xT = np.ascontiguousarray(x[c * DPC : (c + 1) * DPC].T.astype(np.float16))
        inA.append({"xT": xT, "Wf": Wf1, "bf": bf1})
    resA = bass_utils.run_bass_kernel_spmd(ncA, inA, core_ids=cores)
    last_results.append(resA)
    wh = np.concatenate(
        [resA.results[c]["outT"][:F].T.astype(np.float32) for c in cores], axis=0
    )
    s_all = np.concatenate(
        [resA.results[c]["outT"][F : F + 2].astype(np.float32) for c in cores], axis=1
    )
    si_full, sj_full = s_all[0], s_all[1]

    key = ("B", sched.n_tiles, sched.w_total, tuple(sched.calls))
    if key not in _cache:
        _cache[key] = build_progB(sched)
    ncB = _cache[key]

    def launch_B(wh_full, si_f, sj_f, bA, Wn, bWn, An):
        wh16 = wh_full.astype(np.float16)
        Wfn = np.concatenate([Wn, Wn @ An], axis=1).astype(np.float16)
        bfn = np.concatenate(
            [bWn.reshape(F, 1), An.T @ bWn.reshape(F, 1)], axis=0
        ).astype(np.float32)
        inB = []
        for c in cores:
            perm = sched.perms[c]
            real = perm >= 0
            gids = c * DPC + perm[real]
            tmp = np.zeros(sched.n_tiles * 128, np.float32)
            tmp[real] = si_f[gids]
            si_arr = tmp.reshape(sched.n_tiles, 128).T
            esrc = sched.esrc[c]
            whe = wh16[esrc.ravel()].reshape(128, sched.w_total * F)
            sje = np.where(
                sched.emask[c] < 0.0, np.float32(NEG_BIG), sj_f[esrc]
            ).astype(np.float32)
            inB.append(
                {
                    "whe": whe,
                    "sje": sje,
                    "si": np.ascontiguousarray(si_arr),
                    "bA": np.full((128, 1), bA.reshape(-1)[0], np.float32),
                    "flags": sched.flags[c],
                    "Wf": Wfn,
                    "bf": bfn,
                }
            )
        res = bass_utils.run_bass_kernel_spmd(ncB, inB, core_ids=cores)
        last_results.append(res)
        whn = np.zeros((N_NODES, F), np.float32)
        sn_i = np.zeros(N_NODES, np.float32)
        sn_j = np.zeros(N_NODES, np.float32)
        for c in cores:
            perm = sched.perms[c]
            real = perm >= 0
            gids = c * DPC + perm[real]
            outT = res.results[c]["outT"].astype(np.float32)
            whn[gids] = outT[:F].T[real]
            sn_i[gids] = outT[F][real]
            sn_j[gids] = outT[F + 1][real]
        return whn, sn_i, sn_j

    As2 = np.ascontiguousarray(np.concatenate([A2[:F], A2[F:]], axis=1))
    wh2, si2, sj2 = launch_B(wh, si_full, sj_full, bA1, W2, bW2, As2)
    out, _, _ = launch_B(wh2, si2, sj2, bA2, Wfc, bfc, np.zeros((F, 2), np.float32))
    return out.astype(np.float32)


# revision 52
# speedup vs baseline: 1.0539x; 1.0089x over previous
"""GAT (2-layer) on 8 NeuronCores — Bass/Tile kernel.

Strategy (dst-sharded graph parallel):
  - Each core owns 12500 destination nodes, degree-sorted and tiled into
    128-dst tiles on a dense slot grid (slot width = exact per-tile max
    degree over all cores); adjacent similar-degree tiles are merged into
    <=128-column calls (Dmax/Dmin <= 1.25 slack).
  - Host pre-pass is index-only: the slot grid, per-slot source-node ids
    (esrc), pad masks, permutations.
  - Launch A: per-core [Wh1^T; s_i; s_j] = ([W1 | W1@A1]^T x^T + bias) in a
    single fused fp16 matmul per 512-column chunk (f32 accumulate).
  - Between launches the host stages per-core inputs by pure indexing of
    device-computed values: slot-expanded fp16 features whe = Wh16[esrc] and
    pad-masked source scalars sje, so the device streams edge data with
    large sequential DMAs at full HBM bandwidth instead of per-edge gather
    descriptors.
  - Launch B (x2, one per GAT layer), phase 1 computes every attention
    weight up front: e = leaky(s_i + s_j + bA) on DVE+ACT over the whole
    slot grid, ex = exp(e) in fp16 (scores are O(5), no max-subtract
    needed), per-call denominator reduce, reciprocal x zero-degree flags.
    Phase 2 streams calls through a software pipeline: slot-chunk DMA ->
    message multiply split ~50/50 between the gpsimd and vector engines ->
    fp16 pairwise-tree slot reduction (2x DVE mode) -> per-tile scalar-engine
    Prelu fusing the softmax normalization (scale=1/den) with the leaky
    relu -> fp16 PE transpose -> fused epilogue matmul [Wn | Wn@An] giving
    the next layer's [Wh^T; s_i; s_j] (or the final fc output for layer 2).
"""

import dataclasses
import numpy as np

import concourse.bacc as bacc
import concourse.tile as tile
from concourse import bass, mybir, bass_utils
from concourse.masks import make_identity

F32 = mybir.dt.float32
F16 = mybir.dt.float16

N_NODES = 100000
N_CORES = 8
DPC = N_NODES // N_CORES
F = 64
IN_C = 128
NSUB = 1
CALL_W = 128  # max slot-columns per vector-op call
MERGE_SLACK = 1.25  # max Dmax/Dmin when merging tiles into one call
POOL_MULT_SHARE = 0.5  # fraction of message-multiply elems on gpsimd
POOL_TAPER = 0.1  # extra gpsimd share on the last POOL_TAPER_N calls
POOL_TAPER_N = 4
NEG_BIG = -1.0e30
EPS = 1e-16
ALPHA = 0.2


@dataclasses.dataclass
class Schedule:
    n_tiles: int
    w_total: int
    calls: list  # (sub, t0, ntc, D, col0)
    tilecol: np.ndarray  # int32 [w_total]: tile index of each slot column
    perms: list  # per core: int64 [n_tiles*128], local dst or -1
    esrc: list  # per core: int32 [128, w_total] global source id per slot (0 pad)
    emask: list  # per core: f32 [128, w_total] (0 real / NEG_BIG pad)
    flags: list  # per core: f32 [128, n_tiles]


def build_schedule(edge_index: np.ndarray) -> Schedule:
    src = np.asarray(edge_index[0], dtype=np.int64)
    dst = np.asarray(edge_index[1], dtype=np.int64)
    order = np.argsort(dst, kind="stable")
    src_s = src[order]
    deg_all = np.bincount(dst, minlength=N_NODES)
    starts_all = np.concatenate([[0], np.cumsum(deg_all)])

    # per-core sub-shard dst lists (round-robin over degree-sorted order)
    core_subs = []  # [core][sub] -> local dst ids
    for c in range(N_CORES):
        deg = deg_all[c * DPC : (c + 1) * DPC]
        rank = np.argsort(deg, kind="stable")
        core_subs.append([rank[s::NSUB] for s in range(NSUB)])

    # shared tile plan: per (sub, tile): D = max over cores of tile max-deg
    tiles = []
    for s in range(NSUB):
        nt = -(-max(len(core_subs[c][s]) for c in range(N_CORES)) // 128)
        for t in range(nt):
            mx = 1
            for c in range(N_CORES):
                lst = core_subs[c][s][t * 128 : (t + 1) * 128]
                if len(lst):
                    deg = deg_all[c * DPC + lst]
                    mx = max(mx, int(deg.max()))
            assert mx <= 512, mx  # a tile above CALL_W just becomes its own call
            tiles.append((s, mx))
    n_tiles = len(tiles)

    # call plan: merge consecutive same-sub tiles with similar D
    calls = []
    i = 0
    col = 0
    while i < n_tiles:
        s, D = tiles[i]
        Dmax = Dmin = D
        ntc = 1
        while i + ntc < n_tiles:
            s2, D2 = tiles[i + ntc]
            if s2 != s:
                break
            nD, mD = max(Dmax, D2), min(Dmin, D2)
            if (ntc + 1) * nD > CALL_W or nD > MERGE_SLACK * mD:
                break
            Dmax, Dmin = nD, mD
            ntc += 1
        calls.append((s, i, ntc, Dmax, col))
        col += ntc * Dmax
        i += ntc
    w_total = col
    tilecol = np.zeros(w_total, np.int32)
    for (s_, t0, ntc, D, col0) in calls:
        for tl in range(ntc):
            tilecol[col0 + tl * D : col0 + (tl + 1) * D] = t0 + tl

    perms, esrcs, emasks, flagss = [], [], [], []
    for c in range(N_CORES):
        perm = np.full(n_tiles * 128, -1, dtype=np.int64)
        ti = 0
        for s in range(NSUB):
            nt = sum(1 for (ss, _) in tiles if ss == s)
            lst = core_subs[c][s]
            block = np.full(nt * 128, -1, dtype=np.int64)
            block[: len(lst)] = lst
            perm[ti * 128 : (ti + nt) * 128] = block
            ti += nt

        esrc = np.zeros((128, w_total), np.int32)
        emask = np.full((128, w_total), np.float32(NEG_BIG))
        for (s, t0, ntc, D, col0) in calls:
            for tl in range(ntc):
                tglob = t0 + tl
                dsts = perm[tglob * 128 : (tglob + 1) * 128]
                for p in range(128):
                    d = dsts[p]
                    if d < 0:
                        continue
                    g = c * DPC + d
                    e0, ne = starts_all[g], deg_all[g]
                    c0 = col0 + tl * D
                    esrc[p, c0 : c0 + ne] = src_s[e0 : e0 + ne]
                    emask[p, c0 : c0 + ne] = 0.0
        pflat = perm.copy()
        okdeg = (pflat >= 0) & (deg_all[np.clip(c * DPC + pflat, 0, N_NODES - 1)] > 0)
        flags = np.ascontiguousarray(
            okdeg.reshape(n_tiles, 128).T.astype(np.float32)
        )
        perms.append(perm)
        esrcs.append(esrc)
        emasks.append(emask)
        flagss.append(flags)

    return Schedule(n_tiles, w_total, calls, tilecol, perms, esrcs, emasks, flagss)


# ---------------------------------------------------------------- prog A
def build_progA(n_loc=DPC, in_c=IN_C, f=F):
    nc = bacc.Bacc("TRN2", target_bir_lowering=False, debug=False, num_devices=N_CORES)
    xT = nc.dram_tensor("xT", [in_c, n_loc], F16, kind="ExternalInput").ap()
    Wf = nc.dram_tensor("Wf", [in_c, f + 2], F16, kind="ExternalInput").ap()
    bf = nc.dram_tensor("bf", [f + 2, 1], F32, kind="ExternalInput").ap()
    outT = nc.dram_tensor("outT", [f + 2, n_loc], F16, kind="ExternalOutput").ap()

    AF = mybir.ActivationFunctionType
    OP = mybir.AluOpType

    with tile.TileContext(nc) as tc:
        with tc.tile_pool(name="sb", bufs=1) as pool, tc.tile_pool(
            name="ps", bufs=4, space="PSUM"
        ) as pps, tc.tile_pool(name="sb2", bufs=4) as pool2:
            xT_sb = pool.tile([in_c, n_loc], F16)
            Wf_sb = pool.tile([in_c, f + 2], F16)
            nc.sync.dma_start(out=Wf_sb[:], in_=Wf[:, :])
            bf_sb = pool.tile([f + 2, 1], F32)
            nc.sync.dma_start(out=bf_sb[:], in_=bf[:, :])
            NSPL = 12
            spl = -(-n_loc // NSPL)
            for k in range(NSPL):
                a, b = k * spl, min(n_loc, (k + 1) * spl)
                nc.sync.dma_start(out=xT_sb[:, a:b], in_=xT[:, a:b])

            CH = 512
            GB = 6  # chunks per output DMA
            ob = None
            nch = -(-n_loc // CH)
            for ci, c0 in enumerate(range(0, n_loc, CH)):
                ch = min(CH, n_loc - c0)
                ps_w = pps.tile([f + 2, CH], F32, space="PSUM")
                nc.tensor.matmul(
                    out=ps_w[:, :ch],
                    lhsT=Wf_sb[:],
                    rhs=xT_sb[:, c0 : c0 + ch],
                    start=True,
                    stop=True,
                )
                g = ci % GB
                if g == 0:
                    ob = pool2.tile([f + 2, GB * CH], F16, tag="ob")
                if ci % 2 == 0:
                    nc.scalar.activation(
                        out=ob[:, g * CH : g * CH + ch],
                        in_=ps_w[:, :ch],
                        func=AF.Identity,
                        bias=bf_sb[:],
                    )
                else:
                    nc.vector.tensor_scalar(
                        out=ob[:, g * CH : g * CH + ch],
                        in0=ps_w[:, :ch],
                        scalar1=bf_sb[:],
                        scalar2=None,
                        op0=OP.add,
                    )
                if g == GB - 1 or ci == nch - 1:
                    b0 = (ci - g) * CH
                    nc.sync.dma_start(
                        out=outT[:, b0 : c0 + ch], in_=ob[:, : g * CH + ch]
                    )
    nc.compile()
    return nc


# ---------------------------------------------------------------- prog B
def build_progB(sched: Schedule, f=F):
    NT = sched.n_tiles
    WTOT = sched.w_total
    nc = bacc.Bacc(
        "TRN2",
        target_bir_lowering=False,
        debug=False,
        num_devices=N_CORES,
    )
    whe_d = nc.dram_tensor("whe", [128, WTOT * f], F16, kind="ExternalInput").ap()
    sje_d = nc.dram_tensor("sje", [128, WTOT], F32, kind="ExternalInput").ap()
    si_d = nc.dram_tensor("si", [128, NT], F32, kind="ExternalInput").ap()
    bA_d = nc.dram_tensor("bA", [128, 1], F32, kind="ExternalInput").ap()
    flags_d = nc.dram_tensor("flags", [128, NT], F32, kind="ExternalInput").ap()
    Wf_d = nc.dram_tensor("Wf", [f, f + 2], F16, kind="ExternalInput").ap()
    bf_d = nc.dram_tensor("bf", [f + 2, 1], F32, kind="ExternalInput").ap()
    outT_d = nc.dram_tensor("outT", [f + 2, NT * 128], F16, kind="ExternalOutput").ap()

    X = mybir.AxisListType.X
    AF = mybir.ActivationFunctionType
    OP = mybir.AluOpType

    def v(ap, dims, off=0):
        return dataclasses.replace(
            ap,
            ap=[list(ap.ap[0])] + [list(d) for d in dims],
            offset=ap.offset + off,
        )

    with tile.TileContext(nc) as tc:
        with tc.tile_pool(name="const", bufs=1) as pc, tc.tile_pool(
            name="io", bufs=4
        ) as pio, tc.tile_pool(name="work", bufs=4) as pw, tc.tile_pool(
            name="ps", bufs=2, space="PSUM"
        ) as pps, tc.tile_pool(name="ep", bufs=2) as pep:
            # critical-path consts first on the SP DMA queue, then the first
            # call's whe (Pool's multiply slice first so it can start while
            # the rest streams), then the consts only needed later
            sje_sb = pc.tile([128, WTOT], F32)
            nc.sync.dma_start(out=sje_sb[:], in_=sje_d[:, :])
            si_sb = pc.tile([128, NT], F32)
            nc.sync.dma_start(out=si_sb[:], in_=si_d[:, :])
            bA_sb = pc.tile([128, 1], F32)
            nc.sync.dma_start(out=bA_sb[:], in_=bA_d[:, :])
            (s0, t00, ntc0, D0, col00) = sched.calls[0]
            W0 = ntc0 * D0
            sh0 = POOL_MULT_SHARE + (
                POOL_TAPER if len(sched.calls) <= POOL_TAPER_N else 0.0
            )
            cp0 = max(1, min(W0 - 1, int(round(sh0 * W0))))
            whe0 = pio.tile([128, W0 * f], F16, tag="whe")
            nc.sync.dma_start(
                out=whe0[:, : cp0 * f], in_=whe_d[:, col00 * f : (col00 + cp0) * f]
            )
            nc.sync.dma_start(
                out=whe0[:, cp0 * f :],
                in_=whe_d[:, (col00 + cp0) * f : (col00 + W0) * f],
            )
            flags_sb = pc.tile([128, NT], F32)
            nc.sync.dma_start(out=flags_sb[:], in_=flags_d[:, :])
            Wf_sb = pc.tile([f, f + 2], F16)
            nc.sync.dma_start(out=Wf_sb[:], in_=Wf_d[:, :])
            bf_sb = pc.tile([f + 2, 1], F32)
            nc.sync.dma_start(out=bf_sb[:], in_=bf_d[:, :])
            ident = pc.tile([128, 128], F16)
            make_identity(nc, ident[:])

            # ---- phase 1: attention weights for the whole slot grid ----
            # e = leaky(si + sj + bA) (sj pre-masked to -inf on pad slots);
            # ex = exp(e)  (no max-subtract: scores are O(10) so exp fits
            # fp16/f32 comfortably)
            ep_sb = pc.tile([128, WTOT], F32)
            for (s, t0, ntc, D, col0) in sched.calls:
                nc.vector.tensor_tensor(
                    out=v(ep_sb[:], [(D, ntc), (1, D)], off=col0),
                    in0=v(sje_sb[:], [(D, ntc), (1, D)], off=col0),
                    in1=si_sb[:, t0 : t0 + ntc].to_broadcast([128, ntc, D]),
                    op=OP.add,
                )
            ex16 = pc.tile([128, WTOT], F16)
            nc.scalar.activation(
                out=ep_sb[:], in_=ep_sb[:], func=AF.Prelu, alpha=ALPHA, bias=bA_sb[:]
            )
            nc.scalar.activation(out=ex16[:], in_=ep_sb[:], func=AF.Exp)
            den = pc.tile([128, NT], F32)
            rden = pc.tile([128, NT], F32)

            def denominators():
                for (s, t0, ntc, D, col0) in sched.calls:
                    nc.vector.tensor_reduce(
                        out=den[:, t0 : t0 + ntc],
                        in_=v(ex16[:], [(D, ntc), (1, D)], off=col0),
                        axis=X,
                        op=OP.add,
                    )
                nc.vector.tensor_scalar(
                    out=den[:], in0=den[:], scalar1=EPS, scalar2=None, op0=OP.add
                )
                nc.vector.reciprocal(out=rden[:], in_=den[:])
                nc.vector.tensor_tensor(
                    out=rden[:], in0=rden[:], in1=flags_sb[:], op=OP.mult
                )

            # ---- phase 2: stream messages, reduce, epilogue ----
            CHT = 8  # tiles per epilogue chunk (1024 dsts)

            def flush_chunk(ts, ntl, hTL, ob):
                cols = ntl * 128
                ps_w = pps.tile([f + 2, CHT * 128], F32, tag="psw", space="PSUM")
                for q0 in range(0, cols, 512):
                    qw = min(512, cols - q0)
                    nc.tensor.matmul(
                        out=ps_w[:, q0 : q0 + qw],
                        lhsT=Wf_sb[:],
                        rhs=hTL[:, q0 : q0 + qw],
                        start=True,
                        stop=True,
                    )
                nc.scalar.activation(
                    out=ob[:, :cols],
                    in_=ps_w[:, :cols],
                    func=AF.Identity,
                    bias=bf_sb[:],
                )
                nc.sync.dma_start(
                    out=outT_d[:, ts * 128 : ts * 128 + cols],
                    in_=ob[:, :cols],
                )

            def stage1(ci, whe=None):
                (s, t0, ntc, D, col0) = sched.calls[ci]
                W = ntc * D
                sh = POOL_MULT_SHARE + (
                    POOL_TAPER if ci >= len(sched.calls) - POOL_TAPER_N else 0.0
                )
                cp = max(1, min(W - 1, int(round(sh * W))))
                if whe is None:
                    whe = pio.tile([128, W * f], F16, tag="whe")
                    nc.sync.dma_start(
                        out=whe[:], in_=whe_d[:, col0 * f : (col0 + W) * f]
                    )
                # weighted message (fp16): msg = Wh16 * ex16; each call's
                # multiply is split between gpsimd and DVE at the balance point
                msg16 = pw.tile([128, W * f], F16, tag="msg16")
                nc.gpsimd.tensor_tensor(
                    out=v(msg16[:], [(f, cp), (1, f)]),
                    in0=v(whe[:], [(f, cp), (1, f)]),
                    in1=ex16[:, col0 : col0 + cp].to_broadcast([128, cp, f]),
                    op=OP.mult,
                )
                nc.vector.tensor_tensor(
                    out=v(msg16[:], [(f, W - cp), (1, f)], off=cp * f),
                    in0=v(whe[:], [(f, W - cp), (1, f)], off=cp * f),
                    in1=ex16[:, col0 + cp : col0 + W].to_broadcast(
                        [128, W - cp, f]
                    ),
                    op=OP.mult,
                )
                return msg16

            def stage2(ci, msg16):
                (s, t0, ntc, D, col0) = sched.calls[ci]
                W = ntc * D
                # pairwise-tree reduce over the D slots (fp16 packed -> 2x DVE)
                hraw = pw.tile([128, ntc * f], F32, tag="hraw")
                if D == 1:
                    nc.vector.tensor_copy(
                        out=v(hraw[:], [(f, ntc), (1, f)]),
                        in_=v(msg16[:], [(D * f, ntc), (1, f)]),
                    )
                else:
                    p2 = 1
                    while p2 * 2 <= D:
                        p2 *= 2
                    if D > p2:
                        r = D - p2
                        nc.vector.tensor_tensor(
                            out=v(msg16[:], [(D * f, ntc), (1, r * f)]),
                            in0=v(msg16[:], [(D * f, ntc), (1, r * f)]),
                            in1=v(msg16[:], [(D * f, ntc), (1, r * f)], off=p2 * f),
                            op=OP.add,
                        )
                    while p2 > 2:
                        h = p2 // 2
                        nc.vector.tensor_tensor(
                            out=v(msg16[:], [(D * f, ntc), (1, h * f)]),
                            in0=v(msg16[:], [(D * f, ntc), (1, h * f)]),
                            in1=v(msg16[:], [(D * f, ntc), (1, h * f)], off=h * f),
                            op=OP.add,
                        )
                        p2 = h
                    nc.vector.tensor_tensor(
                        out=v(hraw[:], [(f, ntc), (1, f)]),
                        in0=v(msg16[:], [(D * f, ntc), (1, f)]),
                        in1=v(msg16[:], [(D * f, ntc), (1, f)], off=f),
                        op=OP.add,
                    )
                # h = leaky(hraw * rden): per-tile ACT fuses the softmax
                # normalization (scale) with the leaky relu; the last two
                # calls run it on the (by then idle) DVE to shorten the tail
                tail_call = ci >= len(sched.calls) - 2
                if tail_call:
                    nc.vector.tensor_tensor(
                        out=v(hraw[:], [(f, ntc), (1, f)]),
                        in0=v(hraw[:], [(f, ntc), (1, f)]),
                        in1=rden[:, t0 : t0 + ntc].to_broadcast([128, ntc, f]),
                        op=OP.mult,
                    )
                    hl_all = pw.tile([128, ntc * f], F16, tag="hl_all")
                    nc.vector.scalar_tensor_tensor(
                        out=hl_all[:],
                        in0=hraw[:],
                        scalar=ALPHA,
                        in1=hraw[:],
                        op0=OP.mult,
                        op1=OP.max,
                    )
                ps_chunk = None
                for tl in range(ntc):
                    t = t0 + tl
                    if tail_call:
                        hl = hl_all[:, tl * f : (tl + 1) * f]
                    else:
                        hlt = pw.tile([128, f], F16, tag="hl")
                        nc.scalar.activation(
                            out=hlt[:],
                            in_=hraw[:, tl * f : (tl + 1) * f],
                            func=AF.Prelu,
                            alpha=ALPHA,
                            scale=rden[:, t : t + 1],
                        )
                        hl = hlt[:]
                    j = tl % CHT
                    if j == 0:
                        ps_chunk = pps.tile(
                            [f, CHT * 128], F16, tag="pst", space="PSUM"
                        )
                        hTL = pep.tile([f, CHT * 128], F16, tag="hTL")
                        ob = pep.tile([f + 2, CHT * 128], F16, tag="ob")
                    # transposes accumulate into one PSUM chunk; a single
                    # copy then moves the whole chunk to SBUF for the matmul
                    nc.tensor.transpose(
                        out=ps_chunk[:, j * 128 : (j + 1) * 128],
                        in_=hl,
                        identity=ident[:],
                    )
                    if j == CHT - 1 or tl == ntc - 1:
                        cols = (j + 1) * 128
                        nc.scalar.activation(
                            out=hTL[:, :cols],
                            in_=ps_chunk[:, :cols],
                            func=AF.Identity,
                        )
                        flush_chunk(t - j, j + 1, hTL, ob)

            # 2-stage software pipeline: issue call ci+1's DMA+multiplies
            # before call ci's reduce/epilogue so the in-order DVE queue
            # never blocks the next multiply behind a Pool-gated reduce
            ncalls = len(sched.calls)
            prev = stage1(0, whe0)
            denominators()  # after call 0's multiply so DVE starts it sooner
            for ci in range(1, ncalls):
                cur = stage1(ci)
                stage2(ci - 1, prev)
                prev = cur
            stage2(ncalls - 1, prev)
    nc.compile()
    return nc


# ---------------------------------------------------------------- driver
_cache = {}
last_results = []  # BassKernelResults per launch (for test.py profiling)


def kernel(x, edge_index, W1, bW1, A1, bA1, W2, bW2, A2, bA2, Wfc, bfc):
    x = np.asarray(x, dtype=np.float32)
    edge_index = np.asarray(edge_index)
    W1 = np.asarray(W1, np.float32)
    bW1 = np.asarray(bW1, np.float32)
    A1 = np.asarray(A1, np.float32)
    bA1 = np.asarray(bA1, np.float32)
    W2 = np.asarray(W2, np.float32)
    bW2 = np.asarray(bW2, np.float32)
    A2 = np.asarray(A2, np.float32)
    bA2 = np.asarray(bA2, np.float32)
    Wfc = np.asarray(Wfc, np.float32)
    bfc = np.asarray(bfc, np.float32)

    sched = build_schedule(edge_index)
    cores = list(range(N_CORES))
    last_results.clear()

    if "A" not in _cache:
        _cache["A"] = build_progA()
    ncA = _cache["A"]
    As1 = np.ascontiguousarray(np.concatenate([A1[:F], A1[F:]], axis=1))
    Wf1 = np.concatenate([W1, W1 @ As1], axis=1).astype(np.float16)
    bf1 = np.concatenate(
        [bW1.reshape(F, 1), As1.T @ bW1.reshape(F, 1)], axis=0
    ).astype(np.float32)
    inA = []
    for c in cores:
        xT = np.ascontiguousarray(x[c * DPC : (c + 1) * DPC].T.astype(np.float16))
        inA.append({"xT": xT, "Wf": Wf1, "bf": bf1})
    resA = bass_utils.run_bass_kernel_spmd(ncA, inA, core_ids=cores)
    last_results.append(resA)
    wh = np.concatenate(
        [resA.results[c]["outT"][:F].T.astype(np.float32) for c in cores], axis=0
    )
    s_all = np.concatenate(
        [resA.results[c]["outT"][F : F + 2].astype(np.float32) for c in cores], axis=1
    )
    si_full, sj_full = s_all[0], s_all[1]

    key = ("B", sched.n_tiles, sched.w_total, tuple(sched.calls))
    if key not in _cache:
        _cache[key] = build_progB(sched)
    ncB = _cache[key]

    def launch_B(wh_full, si_f, sj_f, bA, Wn, bWn, An):
        wh16 = wh_full.astype(np.float16)
        Wfn = np.concatenate([Wn, Wn @ An], axis=1).astype(np.float16)
        bfn = np.concatenate(
            [bWn.reshape(F, 1), An.T @ bWn.reshape(F, 1)], axis=0
        ).astype(np.float32)
        inB = []
        for c in cores:
            perm = sched.perms[c]
            real = perm >= 0
            gids = c * DPC + perm[real]
            tmp = np.zeros(sched.n_tiles * 128, np.float32)
            tmp[real] = si_f[gids]
            si_arr = tmp.reshape(sched.n_tiles, 128).T
            esrc = sched.esrc[c]
            whe = wh16[esrc.ravel()].reshape(128, sched.w_total * F)
            sje = np.where(
                sched.emask[c] < 0.0, np.float32(NEG_BIG), sj_f[esrc]
            ).astype(np.float32)
            inB.append(
                {
                    "whe": whe,
                    "sje": sje,
                    "si": np.ascontiguousarray(si_arr),
                    "bA": np.full((128, 1), bA.reshape(-1)[0], np.float32),
                    "flags": sched.flags[c],
                    "Wf": Wfn,
                    "bf": bfn,
                }
            )
        res = bass_utils.run_bass_kernel_spmd(ncB, inB, core_ids=cores)
        last_results.append(res)
        whn = np.zeros((N_NODES, F), np.float32)
        sn_i = np.zeros(N_NODES, np.float32)
        sn_j = np.zeros(N_NODES, np.float32)
        for c in cores:
            perm = sched.perms[c]
            real = perm >= 0
            gids = c * DPC + perm[real]
            outT = res.results[c]["outT"].astype(np.float32)
            whn[gids] = outT[:F].T[real]
            sn_i[gids] = outT[F][real]
            sn_j[gids] = outT[F + 1][real]
        return whn, sn_i, sn_j

    As2 = np.ascontiguousarray(np.concatenate([A2[:F], A2[F:]], axis=1))
    wh2, si2, sj2 = launch_B(wh, si_full, sj_full, bA1, W2, bW2, As2)
    out, _, _ = launch_B(wh2, si2, sj2, bA2, Wfc, bfc, np.zeros((F, 2), np.float32))
    return out.astype(np.float32)
